# revision 21
# baseline (speedup 1.0000x reference)
"""GraphSAGE 2-layer GNN, fully on-device on 8 Trainium2 NeuronCores.

Node-parallel sharding: dst nodes are greedily bin-packed into 800
blocks of 128 slots (100 blocks per core) so every block carries a
near-equal number of edges (~2000).  Messages are fetched with the
SWDGE `dma_gather` custom op (mlp gpsimd library): ONE instruction
per pair of blocks moves ~4200 rows (single_packet=False), instead of
one 128-row indirect DMA per 128 edges -- the per-instruction SWDGE
fixed cost dominated the old kernel.

dma_gather indices are int16, so the feature table is packed as QUADS:
[25600, 512] fp8(e3m4) rows holding 4 nodes each; an edge fetches its
src's quad (512B) and the matmul reads the right 128B quarter.  To
keep one matmul per 128-edge group, each block's edges are grouped by
their src's quad sub-slot ("color" = local-slot % 4), and a host-side
greedy assigns node colors to balance the per-(block, color) edge
counts; the per-block group->color pattern is shared by all 8 cores
(SPMD) and baked into the program.

Segment-mean aggregation on the tensor engine: per 128-edge group an
indicator ind[e, n] = (slot(dst[e]) == n) * (1/deg(dst[e])) is built
with ONE fused DVE tensor_scalar (in0 = stride-1 iota tile, scalar1 =
dst-slot column, scalar2 = 1/deg column, is_equal + mult -- 4x DVE
perf mode), then psum[feat, slot] += matmul(lhsT=msg quarter (fp8),
rhs=ind (bf16)).  PSUM drains and ReLUs run on the ACT engine.
Layer-1 h is produced in both orientations: node-major fp8 rows
written per block to HBM (the layer-2 message table, AllGather'd in 5
chunks; the tables are f32-typed because the runtime mishandles
1-byte collectives -- producers/consumers bitcast) and feat-major
bf16 in SBUF as the exact layer-2 self term.  Output heads are tiny
matmuls per block; lo/hi rows accumulate in SBUF, written back once
per 20-block chunk.

One Bacc program, compiled once, SPMD on cores 0-7; all
data-dependent quantities (gather rows, dst slots, 1/deg, per-block
color patterns) are inputs / compile keys.  A pure-numpy fallback is
kept in case the device path raises."""

import heapq
import os
import numpy as np
import ml_dtypes

BF16 = ml_dtypes.bfloat16
FP8 = ml_dtypes.float8_e3m4

# ---------------- configuration ----------------


class Cfg:
    def __init__(self, n_nodes, n_edges, n_cores, chunks, gbatch):
        self.N = n_nodes
        self.E = n_edges
        self.NC = n_cores
        self.D = 128
        self.OWNP = 12800              # node slots per core
        self.BLKS = self.OWNP // 128   # dst blocks per core
        self.NBINS = self.BLKS * n_cores
        self.PADN = self.OWNP * n_cores
        # collective chunk sizes in blocks: small first (starts the
        # serialized collective chain early) and small last (short tail)
        self.CHUNKS = list(chunks)
        assert sum(self.CHUNKS) == self.BLKS
        self.GBATCH = gbatch           # blocks per gather instruction
        assert all(c % gbatch == 0 for c in self.CHUNKS)
        self.NCOLL = len(self.CHUNKS)
        self.CUMB = np.cumsum([0] + self.CHUNKS)      # block boundaries
        self.CUMR = self.CUMB * 128                   # row boundaries


def _parse_chunks():
    s = os.environ.get("KERNEL_CHUNKS", "4,10,18,26,26,16")
    return [int(x) for x in s.split(",")]


CFG = Cfg(n_nodes=100000, n_edges=1600000, n_cores=8,
          chunks=_parse_chunks(),
          gbatch=int(os.environ.get("KERNEL_GBATCH", "2")))

LAST_TRACE = None     # BassKernelResults of the launch (test use)
LAST_LAUNCH_S = None  # wall seconds of the device launch (test use)


def _trace_available():
    try:
        from antenv.axon_hooks import get_axon_ntff_profile_hook  # noqa
        return True
    except Exception:
        return False


# ---------------- host-side edge prep ----------------


def _balance_bins(cfg, deg):
    """Greedy LPT: assign nodes to NBINS bins of <=128 nodes so per-bin
    edge (dst-degree) sums are near-equal.  Returns bin_of[node]."""
    order = np.argsort(-deg, kind="stable")
    bin_of = np.empty(cfg.N, np.int32)
    counts = np.zeros(cfg.NBINS, np.int32)
    heap = [(0, b) for b in range(cfg.NBINS)]
    heapq.heapify(heap)
    for n in order:
        while True:
            s, b = heapq.heappop(heap)
            if counts[b] < 128:
                break
        bin_of[n] = b
        counts[b] += 1
        if counts[b] < 128:
            heapq.heappush(heap, (s + int(deg[n]), b))
    return bin_of


def _color_assign(cfg, src, dst, bin_of):
    """Assign each node a color 0..3 (its quad sub-slot) balancing the
    per-(dst-bin, src-color) edge counts; <=32 nodes per (own-bin,
    color).  Returns c_of[node], slot_of[node] (slot%4 == color), L."""
    order = np.argsort(src, kind="stable")
    bins_o = bin_of[dst[order]]
    srcdeg = np.bincount(src, minlength=cfg.N)
    starts = np.zeros(cfg.N + 1, np.int64)
    starts[1:] = np.cumsum(srcdeg)

    L = np.zeros((cfg.NBINS, 4), np.int64)
    ccount = np.zeros((cfg.NBINS, 4), np.int32)
    c_of = np.full(cfg.N, -1, np.int8)
    slot_of = np.empty(cfg.N, np.int32)

    for n in np.argsort(-srcdeg, kind="stable"):
        own = bin_of[n]
        seg = bins_o[starts[n]:starts[n + 1]]
        if len(seg):
            bs, ks = np.unique(seg, return_counts=True)
            best_c, best_s = -1, None
            for c in range(4):
                if ccount[own, c] >= 32:
                    continue
                sc = np.sum((2 * L[bs, c] + ks) * ks)
                if best_s is None or sc < best_s:
                    best_c, best_s = c, sc
        else:
            best_c = int(np.argmin(
                np.where(ccount[own] < 32, ccount[own], 999)))
        c_of[n] = best_c
        slot_of[n] = 4 * ccount[own, best_c] + best_c
        ccount[own, best_c] += 1
        if len(seg):
            L[bs, best_c] += ks
    return c_of, slot_of, L


def _row_of(cfg, core, li):
    """HBM row of local slot li on core: chunk-major so each collective
    chunk's AllGather output is contiguous (variable chunk sizes)."""
    blk = li // 128
    q = np.searchsorted(cfg.CUMB, blk, side="right") - 1
    rows_q = (cfg.CUMB[q + 1] - cfg.CUMB[q]) * 128
    return cfg.CUMR[q] * cfg.NC + core * rows_q + (li - cfg.CUMR[q])


def _prep(cfg, edge_index, x):
    src = np.asarray(edge_index[0], np.int64)
    dst = np.asarray(edge_index[1], np.int64)

    deg = np.bincount(dst, minlength=cfg.N)
    invc_node = (1.0 / np.maximum(deg, 1)).astype(np.float32)

    bin_of = _balance_bins(cfg, deg)
    c_of, slot_of, L = _color_assign(cfg, src, dst, bin_of)
    core_of = bin_of // cfg.BLKS
    blk_of = bin_of % cfg.BLKS
    li_of = blk_of * 128 + slot_of              # local node slot per core
    row_of = _row_of(cfg, core_of, li_of).astype(np.int64)
    qrow_of = (row_of // 4).astype(np.int16)    # quad row (int16-safe)
    assert row_of.max() < cfg.PADN and (row_of // 4).max() < 32768

    # per-(block, color) group structure, shared across cores
    gcount = -(-L // 128)                       # [NBINS, 4] groups needed
    gshared = np.zeros((cfg.BLKS, 4), np.int64)
    for c in range(4):
        gshared[:, c] = gcount[:, c].reshape(cfg.NC, cfg.BLKS).max(0)
    gshared = np.maximum(gshared, 0)
    scol = []                                    # per blk: color per group
    coff = np.zeros(cfg.BLKS + 1, np.int64)
    cbase = np.zeros((cfg.BLKS, 4), np.int64)
    for blk in range(cfg.BLKS):
        cols = []
        off = coff[blk]
        for c in range(4):
            cbase[blk, c] = off
            cols += [c] * int(gshared[blk, c])
            off += gshared[blk, c]
        scol.append(tuple(cols))
        coff[blk + 1] = off
    totg = int(coff[-1])

    # place edges into (color-segmented) slots of their dst's bin
    be = bin_of[dst]
    ce = c_of[src].astype(np.int64)
    key = be * 4 + ce
    order = np.argsort(key, kind="stable")
    kcounts = np.bincount(key, minlength=cfg.NBINS * 4)
    kstarts = np.zeros(cfg.NBINS * 4, np.int64)
    kstarts[1:] = np.cumsum(kcounts)[:-1]
    rank = np.arange(cfg.E, dtype=np.int64) - kstarts[key[order]]
    so, do = src[order], dst[order]
    beo, ceo = be[order], ce[order]
    core_e = beo // cfg.BLKS
    col = cbase[beo % cfg.BLKS, ceo] + rank // 128
    p = rank % 128

    idx_lin = np.zeros((cfg.NC, totg, 128), np.int16)
    dstl_all = np.full((cfg.NC, 128, totg), -1.0, np.float32)
    invc_all = np.zeros((cfg.NC, 128, totg), np.float32)
    idx_lin[core_e, col, p] = qrow_of[so]
    dstl_all[core_e, p, col] = slot_of[do].astype(np.float32)
    invc_all[core_e, p, col] = invc_node[do]

    # wrap indices per gather instruction: blocks [t*GBATCH, (t+1)*GBATCH);
    # within an instruction, index r -> [r % 16 (replicated x8), r // 16]
    nins = cfg.BLKS // cfg.GBATCH
    woff = np.zeros(nins + 1, np.int64)
    for t in range(nins):
        nidx = int(coff[(t + 1) * cfg.GBATCH] - coff[t * cfg.GBATCH]) * 128
        woff[t + 1] = woff[t] + nidx // 16
    idxw = np.zeros((cfg.NC, 128, int(woff[-1])), np.int16)
    for t in range(nins):
        g0, g1 = int(coff[t * cfg.GBATCH]), int(coff[(t + 1) * cfg.GBATCH])
        seg = idx_lin[:, g0:g1, :].reshape(cfg.NC, -1)   # [NC, nidx]
        nidx = seg.shape[1]
        w = np.zeros((cfg.NC, 16, nidx // 16), np.int16)
        r = np.arange(nidx)
        w[:, r % 16, r // 16] = seg
        idxw[:, :, int(woff[t]):int(woff[t + 1])] = np.tile(w, (1, 8, 1))
    idxw = np.ascontiguousarray(idxw)

    x32 = np.asarray(x, np.float32)
    xpad = np.zeros((cfg.PADN, cfg.D), FP8)
    xpad[row_of] = x32.astype(FP8)
    xpad = np.ascontiguousarray(xpad.reshape(cfg.PADN // 4, cfg.D * 4))
    x_bf = np.ascontiguousarray(x32.astype(BF16))
    xT1 = np.zeros((cfg.NC, cfg.D, cfg.OWNP), BF16)
    for k in range(cfg.NC):
        sel = core_of == k
        xT1[k][:, li_of[sel]] = x_bf[sel].T

    iota128 = np.ascontiguousarray(
        np.broadcast_to(np.arange(128, dtype=np.float32).astype(BF16),
                        (128, 128)))

    return dict(scol=tuple(scol), totg=totg, coff=coff, woff=woff,
                idxw=idxw, dstl_all=dstl_all, invc_all=invc_all,
                xpad=xpad, xT1=xT1, iota128=iota128,
                core_of=core_of, li_of=li_of)


# ---------------- bass program ----------------


def _build(cfg, scol, coff, woff):
    import concourse.bass as bass  # noqa: F401
    import concourse.tile as tile
    import concourse.mybir as mybir
    from concourse import bacc
    from concourse.library_config import mlp

    f32 = mybir.dt.float32
    bf16 = mybir.dt.bfloat16
    fp8 = mybir.dt.float8e3
    i16 = mybir.dt.int16
    AOT = mybir.AluOpType
    ACT_F = mybir.ActivationFunctionType

    D, BLKS, GBAT = cfg.D, cfg.BLKS, cfg.GBATCH
    totg = int(coff[-1])
    wtot = int(woff[-1])

    nc = bacc.Bacc("TRN2", target_bir_lowering=False, debug=False)
    xp_d = nc.dram_tensor("xpad", [cfg.PADN // 4, 4 * D], fp8,
                          kind="ExternalInput")
    xt1_d = nc.dram_tensor("xT1", [D, cfg.OWNP], bf16, kind="ExternalInput")
    idxw_d = nc.dram_tensor("idxw", [128, wtot], i16, kind="ExternalInput")
    dstl_d = nc.dram_tensor("dstl_all", [128, totg], f32,
                            kind="ExternalInput")
    invc_d = nc.dram_tensor("invc_all", [128, totg], f32,
                            kind="ExternalInput")
    iota_d = nc.dram_tensor("iota128", [128, 128], bf16, kind="ExternalInput")
    w_d = {}
    for w in ("wl1", "wr1", "wl2", "wr2"):
        w_d[w] = nc.dram_tensor(w, [D, D], bf16, kind="ExternalInput")
    brow1_d = nc.dram_tensor("brow1", [1, D], bf16, kind="ExternalInput")
    brow2_d = nc.dram_tensor("brow2", [1, D], bf16, kind="ExternalInput")
    ones_d = nc.dram_tensor("ones_r", [1, D], bf16, kind="ExternalInput")
    wpd_d = nc.dram_tensor("wpd", [D, 2], bf16, kind="ExternalInput")
    bpd_d = nc.dram_tensor("bpd", [1, 2], bf16, kind="ExternalInput")
    pd_d = nc.dram_tensor("pd", [2, cfg.OWNP], f32, kind="ExternalOutput")

    # h1 tables hold fp8 bytes but are typed f32: the runtime mishandles
    # 1-byte dtypes in collectives; producers/consumers bitcast.
    h1own = [nc.dram_tensor(f"h1own{q}", [cfg.CHUNKS[q] * 32, D], f32)
             for q in range(cfg.NCOLL)]
    h1g_shared = cfg.NC > 4 and not os.environ.get("KERNEL_H1G_LOCAL")
    h1g = nc.dram_tensor("h1gath", [cfg.PADN // 4, D], f32,
                         addr_space="Shared" if h1g_shared else "Local")

    with tile.TileContext(nc) as tc:
        with (
            tc.tile_pool(name="const", bufs=1) as cp,
            tc.tile_pool(name="msgp", bufs=3) as mp,
            tc.tile_pool(name="indp", bufs=3) as ip,
            tc.tile_pool(name="sbp", bufs=6) as sp,
            tc.tile_pool(name="pdp", bufs=1) as dp,
            tc.tile_pool(name="psa", bufs=2, space="PSUM") as pa_pool,
            tc.tile_pool(name="psh", bufs=2, space="PSUM") as ph_pool,
            tc.tile_pool(name="psh2", bufs=2, space="PSUM") as ph2_pool,
            tc.tile_pool(name="psd", bufs=1, space="PSUM") as pd_pool,
        ):
            nc.gpsimd.load_library(mlp)
            wt = {}
            for w in ("wl1", "wr1", "wl2", "wr2"):
                t = cp.tile([D, D], bf16, tag=w)
                nc.sync.dma_start(t[:], w_d[w][:])
                wt[w] = t
            brow1_t = cp.tile([1, D], bf16, tag="brow1")
            nc.sync.dma_start(brow1_t[:], brow1_d[:])
            brow2_t = cp.tile([1, D], bf16, tag="brow2")
            nc.sync.dma_start(brow2_t[:], brow2_d[:])
            ones_t = cp.tile([1, D], bf16, tag="ones")
            nc.sync.dma_start(ones_t[:], ones_d[:])
            wpd_t = cp.tile([D, 2], bf16, tag="wpd")
            nc.sync.dma_start(wpd_t[:], wpd_d[:])
            bpd_t = cp.tile([1, 2], bf16, tag="bpd")
            nc.sync.dma_start(bpd_t[:], bpd_d[:])
            iota_t = cp.tile([128, 128], bf16, tag="iota")
            nc.sync.dma_start(iota_t[:], iota_d[:])
            idxw_t = cp.tile([128, wtot], i16, tag="idxw")
            nc.sync.dma_start(idxw_t[:], idxw_d[:])
            dstl_t = cp.tile([128, totg], f32, tag="dstl")
            nc.sync.dma_start(dstl_t[:], dstl_d[:])
            invc_t = cp.tile([128, totg], f32, tag="invc")
            nc.sync.dma_start(invc_t[:], invc_d[:])
            xT1_t = cp.tile([D, cfg.OWNP], bf16, tag="xT1")
            nc.sync.dma_start(xT1_t[:], xt1_d[:])
            xT2_t = cp.tile([D, cfg.OWNP], bf16, tag="xT2")

            for layer in (1, 2):
                wl_t = wt["wl1"] if layer == 1 else wt["wl2"]
                wr_t = wt["wr1"] if layer == 1 else wt["wr2"]
                brow_t = brow1_t if layer == 1 else brow2_t
                xTs_t = xT1_t if layer == 1 else xT2_t

                msg = None
                g_base = 0
                lo_sb = hi_sb = None
                for b in range(BLKS):
                    q = int(np.searchsorted(cfg.CUMB, b, side="right") - 1)
                    b0 = int(cfg.CUMB[q])
                    cbq = cfg.CHUNKS[q]
                    if b % GBAT == 0:
                        t = b // GBAT
                        g0, g1 = int(coff[b]), int(coff[b + GBAT])
                        g_base = g0
                        nseg = g1 - g0
                        nidx = nseg * 128
                        msg = mp.tile([128, nseg * 512], fp8, tag="msg")
                        in_ap = xp_d[:] if layer == 1 else \
                            h1g[:].bitcast(fp8)
                        nc.gpsimd.dma_gather(
                            out_ap=msg[:].rearrange("p (g e) -> p g e",
                                                    e=512),
                            in_ap=in_ap,
                            idxs_ap=idxw_t[:, int(woff[t]):int(woff[t + 1])],
                            num_idxs=nidx,
                            num_idxs_reg=nidx,
                            elem_size=512,
                            single_packet=False)

                    cols = scol[b]
                    nbg = len(cols)
                    boff = int(coff[b]) - g_base

                    ind = ip.tile([128, nbg * 128], bf16, tag="ind")
                    for g in range(nbg):
                        c = int(coff[b]) + g
                        nc.vector.tensor_scalar(
                            ind[:, g * 128:(g + 1) * 128],
                            iota_t[:],
                            dstl_t[:, c:c + 1],
                            invc_t[:, c:c + 1],
                            AOT.is_equal, AOT.mult)

                    pa = pa_pool.tile([128, 128], f32, tag="agg")
                    for g in range(nbg):
                        mo = (boff + g) * 512 + cols[g] * 128
                        nc.tensor.matmul(
                            pa[:], msg[:, mo:mo + 128],
                            ind[:, g * 128:(g + 1) * 128],
                            start=(g == 0), stop=(g == nbg - 1))

                    aggsb = sp.tile([128, 128], bf16, tag="aggsb")
                    nc.scalar.activation(aggsb[:], pa[:], ACT_F.Copy,
                                         bias=0.0, scale=1.0)
                    if layer == 1:
                        # node-major h rows -> fp8 table for layer 2
                        ph = ph_pool.tile([128, 128], f32, tag="ph")
                        nc.tensor.matmul(ph[:], aggsb[:], wl_t[:],
                                         start=True, stop=False)
                        nc.tensor.matmul(
                            ph[:], xTs_t[:, b * 128:b * 128 + 128],
                            wr_t[:], start=False, stop=False)
                        nc.tensor.matmul(ph[:], ones_t[:], brow_t[:],
                                         start=False, stop=True)
                        hsb = sp.tile([128, 128], fp8, tag="hsb")
                        nc.scalar.activation(hsb[:], ph[:], ACT_F.Relu,
                                             bias=0.0, scale=1.0)
                        r0 = (b - b0) * 128
                        nc.sync.dma_start(
                            h1own[q][r0 // 4:(r0 + 128) // 4, :],
                            hsb[:].bitcast(f32))
                        # feat-major h -> exact bf16 self term for layer 2
                        ph2 = ph2_pool.tile([128, 128], f32, tag="ph2")
                        nc.tensor.matmul(ph2[:], wl_t[:], aggsb[:],
                                         start=True, stop=False)
                        nc.tensor.matmul(
                            ph2[:], wr_t[:],
                            xTs_t[:, b * 128:b * 128 + 128],
                            start=False, stop=False)
                        nc.tensor.matmul(ph2[:], brow_t[:], ones_t[:],
                                         start=False, stop=True)
                        nc.scalar.activation(
                            xT2_t[:, b * 128:b * 128 + 128], ph2[:],
                            ACT_F.Relu, bias=0.0, scale=1.0)
                        # chunk-q AllGather, emitted one gather-batch late
                        # so its sem wait (holding Pool SEQ) doesn't block
                        # the next chunk's gathers in the Pool queue.
                        def emit_coll(qq):
                            r0 = int(cfg.CUMR[qq]) * cfg.NC // 4
                            r1 = int(cfg.CUMR[qq + 1]) * cfg.NC // 4
                            nc.gpsimd.collective_compute(
                                "AllGather", AOT.bypass,
                                replica_groups=[list(range(cfg.NC))],
                                ins=[h1own[qq][:, :]],
                                outs=[h1g[r0:r1, :]])
                        if b - b0 == GBAT - 1 and q > 0:
                            emit_coll(q - 1)
                        if b == BLKS - 1:
                            emit_coll(cfg.NCOLL - 1)
                    else:
                        if b == b0:
                            lo_sb = dp.tile([1, cbq * 128], f32, tag="lo")
                            hi_sb = dp.tile([1, cbq * 128], f32, tag="hi")
                        off = (b - b0) * 128
                        # h2T = relu(wl.T@agg + wr.T@h1 + b) feat-major
                        ph2 = ph2_pool.tile([128, 128], f32, tag="ph2")
                        nc.tensor.matmul(ph2[:], wl_t[:], aggsb[:],
                                         start=True, stop=False)
                        nc.tensor.matmul(
                            ph2[:], wr_t[:],
                            xTs_t[:, b * 128:b * 128 + 128],
                            start=False, stop=False)
                        nc.tensor.matmul(ph2[:], brow_t[:], ones_t[:],
                                         start=False, stop=True)
                        h2T = sp.tile([128, 128], bf16, tag="h2T")
                        nc.scalar.activation(h2T[:], ph2[:], ACT_F.Relu,
                                             bias=0.0, scale=1.0)
                        pp = pd_pool.tile([1, 128], f32, tag="ppp")
                        nc.tensor.matmul(pp[:], wpd_t[:, 0:1], h2T[:],
                                         start=True, stop=False)
                        nc.tensor.matmul(pp[:], bpd_t[0:1, 0:1], ones_t[:],
                                         start=False, stop=True)
                        zz = pd_pool.tile([1, 128], f32, tag="zzp")
                        nc.tensor.matmul(zz[:], wpd_t[:, 1:2], h2T[:],
                                         start=True, stop=False)
                        nc.tensor.matmul(zz[:], bpd_t[0:1, 1:2], ones_t[:],
                                         start=False, stop=True)
                        drow = sp.tile([1, 128], f32, tag="drow")
                        nc.scalar.activation(drow[:], zz[:], ACT_F.Sigmoid,
                                             bias=0.0, scale=1.0)
                        nc.vector.tensor_sub(
                            lo_sb[0:1, off:off + 128], pp[:], drow[:])
                        nc.vector.tensor_add(
                            hi_sb[0:1, off:off + 128], pp[:], drow[:])
                        if b + 1 == int(cfg.CUMB[q + 1]):
                            c0, c1 = int(cfg.CUMR[q]), int(cfg.CUMR[q + 1])
                            nc.sync.dma_start(pd_d[0:1, c0:c1], lo_sb[:])
                            nc.sync.dma_start(pd_d[1:2, c0:c1], hi_sb[:])
    nc.compile()
    return nc


# ---------------- device path ----------------

_compiled = None       # (key, nc)


def _device_kernel(cfg, x, edge_index, Wl1, Wr1, b1, Wl2, Wr2, b2,
                   Wp, bp, Wd, bd):
    global _compiled, LAST_TRACE, LAST_LAUNCH_S
    import time as _time
    from concourse.bass_utils import run_bass_kernel_spmd

    prep = _prep(cfg, edge_index, x)
    key = prep["scol"]
    if _compiled is None or _compiled[0] != key:
        nc = _build(cfg, prep["scol"], prep["coff"], prep["woff"])
        _compiled = (key, nc)
    nc = _compiled[1]

    def bfw(a):
        return np.ascontiguousarray(np.asarray(a, np.float32).astype(BF16))

    bp_val = float(np.asarray(bp).reshape(-1)[0])
    bd_val = float(np.asarray(bd).reshape(-1)[0])
    wpd = np.stack([np.asarray(Wp, np.float32).reshape(-1),
                    np.asarray(Wd, np.float32).reshape(-1)], 1).astype(BF16)
    bpd = np.array([[bp_val, bd_val]], np.float32).astype(BF16)
    ones_r = np.ones((1, cfg.D), BF16)

    in_maps = []
    for k in range(cfg.NC):
        in_maps.append({
            "xpad": prep["xpad"],
            "xT1": prep["xT1"][k],
            "idxw": prep["idxw"][k],
            "dstl_all": prep["dstl_all"][k],
            "invc_all": prep["invc_all"][k],
            "iota128": prep["iota128"],
            "wl1": bfw(Wl1), "wr1": bfw(Wr1),
            "wl2": bfw(Wl2), "wr2": bfw(Wr2),
            "brow1": bfw(np.asarray(b1).reshape(1, cfg.D)),
            "brow2": bfw(np.asarray(b2).reshape(1, cfg.D)),
            "ones_r": ones_r,
            "wpd": wpd, "bpd": bpd,
        })

    trace = bool(os.environ.get("KERNEL_TRACE")) and _trace_available()
    _t0 = _time.time()
    res = run_bass_kernel_spmd(nc, in_maps, core_ids=list(range(cfg.NC)),
                               trace=trace)
    LAST_LAUNCH_S = _time.time() - _t0
    LAST_TRACE = res
    outs = res.results if hasattr(res, "results") else res
    pd = np.stack([np.asarray(o["pd"], np.float32) for o in outs])
    # unshard: node n lives at (core_of[n], li_of[n])
    core_of, li_of = prep["core_of"], prep["li_of"]
    lo = pd[core_of, 0, li_of].astype(np.float32).reshape(cfg.N, 1)
    hi = pd[core_of, 1, li_of].astype(np.float32).reshape(cfg.N, 1)
    return np.ascontiguousarray(lo), np.ascontiguousarray(hi)


# ---------------- host fallback ----------------


def _host_kernel(x, edge_index, Wl1, Wr1, b1, Wl2, Wr2, b2, Wp, bp, Wd, bd):
    N = CFG.N
    x = np.asarray(x, np.float32)
    src = np.asarray(edge_index[0], np.int64)
    dst = np.asarray(edge_index[1], np.int64)
    order = np.argsort(dst, kind="stable")
    src_s, dst_s = src[order], dst[order]
    counts = np.bincount(dst_s, minlength=N)
    starts = np.zeros(N, np.int64)
    starts[1:] = np.cumsum(counts)[:-1]
    nz = counts > 0
    inv = (1.0 / np.maximum(counts[nz], 1)).astype(np.float32)

    def mean_agg(f):
        sums = np.add.reduceat(f[src_s], starts[nz], axis=0)
        agg = np.zeros((N, f.shape[1]), np.float32)
        agg[nz] = sums * inv[:, None]
        return agg

    def layer(f, Wl, Wr, b):
        return np.maximum(mean_agg(f) @ Wl + f @ Wr + b, 0.0)

    h = layer(x, np.asarray(Wl1, np.float32), np.asarray(Wr1, np.float32),
              np.asarray(b1, np.float32))
    h = layer(h, np.asarray(Wl2, np.float32), np.asarray(Wr2, np.float32),
              np.asarray(b2, np.float32))
    preds = h @ np.asarray(Wp, np.float32) + np.asarray(bp, np.float32)
    z = h @ np.asarray(Wd, np.float32) + np.asarray(bd, np.float32)
    diffs = 1.0 / (1.0 + np.exp(-z))
    return ((preds - diffs).astype(np.float32),
            (preds + diffs).astype(np.float32))


# ---------------- entry ----------------


def kernel(x, edge_index, Wl1, Wr1, b1, Wl2, Wr2, b2, Wp, bp, Wd, bd):
    if not os.environ.get("KERNEL_HOST_ONLY"):
        try:
            return _device_kernel(CFG, x, edge_index, Wl1, Wr1, b1,
                                  Wl2, Wr2, b2, Wp, bp, Wd, bd)
        except Exception:
            import traceback
            traceback.print_exc()
    return _host_kernel(x, edge_index, Wl1, Wr1, b1, Wl2, Wr2, b2,
                        Wp, bp, Wd, bd)


# revision 22
# speedup vs baseline: 1.0494x; 1.0494x over previous
"""GraphSAGE 2-layer GNN, fully on-device on 8 Trainium2 NeuronCores.

Node-parallel sharding: dst nodes are greedily bin-packed into 800
blocks of 128 slots (100 blocks per core) so every block carries a
near-equal number of edges (~2000).  Messages are fetched with the
SWDGE `dma_gather` custom op (mlp gpsimd library): ONE instruction
per pair of blocks moves ~4200 rows (single_packet=False), instead of
one 128-row indirect DMA per 128 edges -- the per-instruction SWDGE
fixed cost dominated the old kernel.

dma_gather indices are int16, so the feature table is packed as QUADS:
[25600, 512] fp8(e3m4) rows holding 4 nodes each; an edge fetches its
src's quad (512B) and the matmul reads the right 128B quarter.  To
keep one matmul per 128-edge group, each block's edges are grouped by
their src's quad sub-slot ("color" = local-slot % 4), and a host-side
greedy assigns node colors to balance the per-(block, color) edge
counts; the per-block group->color pattern is shared by all 8 cores
(SPMD) and baked into the program.

Segment-mean aggregation on the tensor engine: per 128-edge group an
indicator ind[e, n] = (slot(dst[e]) == n) * (1/deg(dst[e])) is built
with ONE fused DVE tensor_scalar (in0 = stride-1 iota tile, scalar1 =
dst-slot column, scalar2 = 1/deg column, is_equal + mult -- 4x DVE
perf mode), then psum[feat, slot] += matmul(lhsT=msg quarter (fp8),
rhs=ind (bf16)).  PSUM drains and ReLUs run on the ACT engine.
Layer-1 h is produced in both orientations: node-major fp8 rows
written per block to HBM (the layer-2 message table, AllGather'd in 5
chunks; the tables are f32-typed because the runtime mishandles
1-byte collectives -- producers/consumers bitcast) and feat-major
bf16 in SBUF as the exact layer-2 self term.  Output heads are tiny
matmuls per block; lo/hi rows accumulate in SBUF, written back once
per 20-block chunk.

One Bacc program, compiled once, SPMD on cores 0-7; all
data-dependent quantities (gather rows, dst slots, 1/deg, per-block
color patterns) are inputs / compile keys.  A pure-numpy fallback is
kept in case the device path raises."""

import heapq
import os
import numpy as np
import ml_dtypes

BF16 = ml_dtypes.bfloat16
FP8 = ml_dtypes.float8_e3m4

# ---------------- configuration ----------------


class Cfg:
    def __init__(self, n_nodes, n_edges, n_cores, chunks, gbatch):
        self.N = n_nodes
        self.E = n_edges
        self.NC = n_cores
        self.D = 128
        self.OWNP = 12800              # node slots per core
        self.BLKS = self.OWNP // 128   # dst blocks per core
        self.NBINS = self.BLKS * n_cores
        self.PADN = self.OWNP * n_cores
        # collective chunk sizes in blocks: small first (starts the
        # serialized collective chain early) and small last (short tail)
        self.CHUNKS = list(chunks)
        assert sum(self.CHUNKS) == self.BLKS
        self.GBATCH = gbatch           # blocks per gather instruction
        assert all(c % gbatch == 0 for c in self.CHUNKS)
        self.NCOLL = len(self.CHUNKS)
        self.CUMB = np.cumsum([0] + self.CHUNKS)      # block boundaries
        self.CUMR = self.CUMB * 128                   # row boundaries


def _parse_chunks():
    s = os.environ.get("KERNEL_CHUNKS", "6,10,14,18,24,28")
    return [int(x) for x in s.split(",")]


CFG = Cfg(n_nodes=100000, n_edges=1600000, n_cores=8,
          chunks=_parse_chunks(),
          gbatch=int(os.environ.get("KERNEL_GBATCH", "2")))

LAST_TRACE = None     # BassKernelResults of the launch (test use)
LAST_LAUNCH_S = None  # wall seconds of the device launch (test use)


def _trace_available():
    try:
        from antenv.axon_hooks import get_axon_ntff_profile_hook  # noqa
        return True
    except Exception:
        return False


# ---------------- host-side edge prep ----------------


def _balance_bins(cfg, deg):
    """Greedy LPT: assign nodes to NBINS bins of <=128 nodes so per-bin
    edge (dst-degree) sums are near-equal.  Returns bin_of[node]."""
    order = np.argsort(-deg, kind="stable")
    bin_of = np.empty(cfg.N, np.int32)
    counts = np.zeros(cfg.NBINS, np.int32)
    heap = [(0, b) for b in range(cfg.NBINS)]
    heapq.heapify(heap)
    for n in order:
        while True:
            s, b = heapq.heappop(heap)
            if counts[b] < 128:
                break
        bin_of[n] = b
        counts[b] += 1
        if counts[b] < 128:
            heapq.heappush(heap, (s + int(deg[n]), b))
    return bin_of


def _color_assign(cfg, src, dst, bin_of):
    """Assign each node a color 0..3 (its quad sub-slot) balancing the
    per-(dst-bin, src-color) edge counts; <=32 nodes per (own-bin,
    color).  Returns c_of[node], slot_of[node] (slot%4 == color), L."""
    order = np.argsort(src, kind="stable")
    bins_o = bin_of[dst[order]]
    srcdeg = np.bincount(src, minlength=cfg.N)
    starts = np.zeros(cfg.N + 1, np.int64)
    starts[1:] = np.cumsum(srcdeg)

    L = np.zeros((cfg.NBINS, 4), np.int64)
    ccount = np.zeros((cfg.NBINS, 4), np.int32)
    c_of = np.full(cfg.N, -1, np.int8)
    slot_of = np.empty(cfg.N, np.int32)

    for n in np.argsort(-srcdeg, kind="stable"):
        own = bin_of[n]
        seg = bins_o[starts[n]:starts[n + 1]]
        if len(seg):
            bs, ks = np.unique(seg, return_counts=True)
            best_c, best_s = -1, None
            for c in range(4):
                if ccount[own, c] >= 32:
                    continue
                sc = np.sum((2 * L[bs, c] + ks) * ks)
                if best_s is None or sc < best_s:
                    best_c, best_s = c, sc
        else:
            best_c = int(np.argmin(
                np.where(ccount[own] < 32, ccount[own], 999)))
        c_of[n] = best_c
        slot_of[n] = 4 * ccount[own, best_c] + best_c
        ccount[own, best_c] += 1
        if len(seg):
            L[bs, best_c] += ks
    return c_of, slot_of, L


def _row_of(cfg, core, li):
    """HBM row of local slot li on core: chunk-major so each collective
    chunk's AllGather output is contiguous (variable chunk sizes)."""
    blk = li // 128
    q = np.searchsorted(cfg.CUMB, blk, side="right") - 1
    rows_q = (cfg.CUMB[q + 1] - cfg.CUMB[q]) * 128
    return cfg.CUMR[q] * cfg.NC + core * rows_q + (li - cfg.CUMR[q])


def _prep(cfg, edge_index, x):
    src = np.asarray(edge_index[0], np.int64)
    dst = np.asarray(edge_index[1], np.int64)

    deg = np.bincount(dst, minlength=cfg.N)
    invc_node = (1.0 / np.maximum(deg, 1)).astype(np.float32)

    bin_of = _balance_bins(cfg, deg)
    c_of, slot_of, L = _color_assign(cfg, src, dst, bin_of)
    core_of = bin_of // cfg.BLKS
    blk_of = bin_of % cfg.BLKS
    li_of = blk_of * 128 + slot_of              # local node slot per core
    row_of = _row_of(cfg, core_of, li_of).astype(np.int64)
    qrow_of = (row_of // 4).astype(np.int16)    # quad row (int16-safe)
    assert row_of.max() < cfg.PADN and (row_of // 4).max() < 32768

    # per-(block, color) group structure, shared across cores
    gcount = -(-L // 128)                       # [NBINS, 4] groups needed
    gshared = np.zeros((cfg.BLKS, 4), np.int64)
    for c in range(4):
        gshared[:, c] = gcount[:, c].reshape(cfg.NC, cfg.BLKS).max(0)
    gshared = np.maximum(gshared, 0)
    scol = []                                    # per blk: color per group
    coff = np.zeros(cfg.BLKS + 1, np.int64)
    cbase = np.zeros((cfg.BLKS, 4), np.int64)
    for blk in range(cfg.BLKS):
        cols = []
        off = coff[blk]
        for c in range(4):
            cbase[blk, c] = off
            cols += [c] * int(gshared[blk, c])
            off += gshared[blk, c]
        scol.append(tuple(cols))
        coff[blk + 1] = off
    totg = int(coff[-1])

    # place edges into (color-segmented) slots of their dst's bin
    be = bin_of[dst]
    ce = c_of[src].astype(np.int64)
    key = be * 4 + ce
    order = np.argsort(key, kind="stable")
    kcounts = np.bincount(key, minlength=cfg.NBINS * 4)
    kstarts = np.zeros(cfg.NBINS * 4, np.int64)
    kstarts[1:] = np.cumsum(kcounts)[:-1]
    rank = np.arange(cfg.E, dtype=np.int64) - kstarts[key[order]]
    so, do = src[order], dst[order]
    beo, ceo = be[order], ce[order]
    core_e = beo // cfg.BLKS
    col = cbase[beo % cfg.BLKS, ceo] + rank // 128
    p = rank % 128

    idx_lin = np.zeros((cfg.NC, totg, 128), np.int16)
    dstl_all = np.full((cfg.NC, 128, totg), -1.0, np.float32)
    invc_all = np.zeros((cfg.NC, 128, totg), np.float32)
    idx_lin[core_e, col, p] = qrow_of[so]
    dstl_all[core_e, p, col] = slot_of[do].astype(np.float32)
    invc_all[core_e, p, col] = invc_node[do]

    # wrap indices per gather instruction: blocks [t*GBATCH, (t+1)*GBATCH);
    # within an instruction, index r -> [r % 16 (replicated x8), r // 16]
    nins = cfg.BLKS // cfg.GBATCH
    woff = np.zeros(nins + 1, np.int64)
    for t in range(nins):
        nidx = int(coff[(t + 1) * cfg.GBATCH] - coff[t * cfg.GBATCH]) * 128
        woff[t + 1] = woff[t] + nidx // 16
    idxw = np.zeros((cfg.NC, 128, int(woff[-1])), np.int16)
    for t in range(nins):
        g0, g1 = int(coff[t * cfg.GBATCH]), int(coff[(t + 1) * cfg.GBATCH])
        seg = idx_lin[:, g0:g1, :].reshape(cfg.NC, -1)   # [NC, nidx]
        nidx = seg.shape[1]
        w = np.zeros((cfg.NC, 16, nidx // 16), np.int16)
        r = np.arange(nidx)
        w[:, r % 16, r // 16] = seg
        idxw[:, :, int(woff[t]):int(woff[t + 1])] = np.tile(w, (1, 8, 1))
    idxw = np.ascontiguousarray(idxw)

    x32 = np.asarray(x, np.float32)
    xpad = np.zeros((cfg.PADN, cfg.D), FP8)
    xpad[row_of] = x32.astype(FP8)
    xpad = np.ascontiguousarray(xpad.reshape(cfg.PADN // 4, cfg.D * 4))
    x_bf = np.ascontiguousarray(x32.astype(BF16))
    xT1 = np.zeros((cfg.NC, cfg.D, cfg.OWNP), BF16)
    for k in range(cfg.NC):
        sel = core_of == k
        xT1[k][:, li_of[sel]] = x_bf[sel].T

    iota128 = np.ascontiguousarray(
        np.broadcast_to(np.arange(128, dtype=np.float32).astype(BF16),
                        (128, 128)))

    return dict(scol=tuple(scol), totg=totg, coff=coff, woff=woff,
                idxw=idxw, dstl_all=dstl_all, invc_all=invc_all,
                xpad=xpad, xT1=xT1, iota128=iota128,
                core_of=core_of, li_of=li_of)


# ---------------- bass program ----------------


def _build(cfg, scol, coff, woff):
    import concourse.bass as bass  # noqa: F401
    import concourse.tile as tile
    import concourse.mybir as mybir
    from concourse import bacc
    from concourse.library_config import mlp

    f32 = mybir.dt.float32
    bf16 = mybir.dt.bfloat16
    fp8 = mybir.dt.float8e3
    i16 = mybir.dt.int16
    AOT = mybir.AluOpType
    ACT_F = mybir.ActivationFunctionType

    D, BLKS, GBAT = cfg.D, cfg.BLKS, cfg.GBATCH
    totg = int(coff[-1])
    wtot = int(woff[-1])

    nc = bacc.Bacc("TRN2", target_bir_lowering=False, debug=False)
    xp_d = nc.dram_tensor("xpad", [cfg.PADN // 4, 4 * D], fp8,
                          kind="ExternalInput")
    xt1_d = nc.dram_tensor("xT1", [D, cfg.OWNP], bf16, kind="ExternalInput")
    idxw_d = nc.dram_tensor("idxw", [128, wtot], i16, kind="ExternalInput")
    dstl_d = nc.dram_tensor("dstl_all", [128, totg], f32,
                            kind="ExternalInput")
    invc_d = nc.dram_tensor("invc_all", [128, totg], f32,
                            kind="ExternalInput")
    iota_d = nc.dram_tensor("iota128", [128, 128], bf16, kind="ExternalInput")
    w_d = {}
    for w in ("wl1", "wr1", "wl2", "wr2"):
        w_d[w] = nc.dram_tensor(w, [D, D], bf16, kind="ExternalInput")
    brow1_d = nc.dram_tensor("brow1", [1, D], bf16, kind="ExternalInput")
    brow2_d = nc.dram_tensor("brow2", [1, D], bf16, kind="ExternalInput")
    ones_d = nc.dram_tensor("ones_r", [1, D], bf16, kind="ExternalInput")
    wpd_d = nc.dram_tensor("wpd", [D, 2], bf16, kind="ExternalInput")
    bpd_d = nc.dram_tensor("bpd", [1, 2], bf16, kind="ExternalInput")
    pd_d = nc.dram_tensor("pd", [2, cfg.OWNP], f32, kind="ExternalOutput")

    # h1 tables hold fp8 bytes but are typed f32: the runtime mishandles
    # 1-byte dtypes in collectives; producers/consumers bitcast.
    h1own = [nc.dram_tensor(f"h1own{q}", [cfg.CHUNKS[q] * 32, D], f32)
             for q in range(cfg.NCOLL)]
    h1g_shared = cfg.NC > 4 and not os.environ.get("KERNEL_H1G_LOCAL")
    h1g = nc.dram_tensor("h1gath", [cfg.PADN // 4, D], f32,
                         addr_space="Shared" if h1g_shared else "Local")

    with tile.TileContext(nc) as tc:
        with (
            tc.tile_pool(name="const", bufs=1) as cp,
            tc.tile_pool(name="msgp", bufs=3) as mp,
            tc.tile_pool(name="indp", bufs=3) as ip,
            tc.tile_pool(name="sbp", bufs=6) as sp,
            tc.tile_pool(name="pdp", bufs=2) as dp,
            tc.tile_pool(name="psa", bufs=2, space="PSUM") as pa_pool,
            tc.tile_pool(name="psh", bufs=2, space="PSUM") as ph_pool,
            tc.tile_pool(name="psh2", bufs=2, space="PSUM") as ph2_pool,
            tc.tile_pool(name="psd", bufs=1, space="PSUM") as pd_pool,
        ):
            nc.gpsimd.load_library(mlp)
            wt = {}
            for w in ("wl1", "wr1", "wl2", "wr2"):
                t = cp.tile([D, D], bf16, tag=w)
                nc.sync.dma_start(t[:], w_d[w][:])
                wt[w] = t
            brow1_t = cp.tile([1, D], bf16, tag="brow1")
            nc.sync.dma_start(brow1_t[:], brow1_d[:])
            brow2_t = cp.tile([1, D], bf16, tag="brow2")
            nc.sync.dma_start(brow2_t[:], brow2_d[:])
            ones_t = cp.tile([1, D], bf16, tag="ones")
            nc.sync.dma_start(ones_t[:], ones_d[:])
            wpd_t = cp.tile([D, 2], bf16, tag="wpd")
            nc.sync.dma_start(wpd_t[:], wpd_d[:])
            bpd_t = cp.tile([1, 2], bf16, tag="bpd")
            nc.sync.dma_start(bpd_t[:], bpd_d[:])
            iota_t = cp.tile([128, 128], bf16, tag="iota")
            nc.sync.dma_start(iota_t[:], iota_d[:])
            idxw_t = cp.tile([128, wtot], i16, tag="idxw")
            nc.sync.dma_start(idxw_t[:], idxw_d[:])
            dstl_t = cp.tile([128, totg], f32, tag="dstl")
            nc.sync.dma_start(dstl_t[:], dstl_d[:])
            invc_t = cp.tile([128, totg], f32, tag="invc")
            nc.sync.dma_start(invc_t[:], invc_d[:])
            xT1_t = cp.tile([D, cfg.OWNP], bf16, tag="xT1")
            nc.sync.dma_start(xT1_t[:], xt1_d[:])
            xT2_t = cp.tile([D, cfg.OWNP], bf16, tag="xT2")

            for layer in (1, 2):
                wl_t = wt["wl1"] if layer == 1 else wt["wl2"]
                wr_t = wt["wr1"] if layer == 1 else wt["wr2"]
                brow_t = brow1_t if layer == 1 else brow2_t
                xTs_t = xT1_t if layer == 1 else xT2_t

                msg = None
                g_base = 0
                lo_sb = hi_sb = None
                for b in range(BLKS):
                    q = int(np.searchsorted(cfg.CUMB, b, side="right") - 1)
                    b0 = int(cfg.CUMB[q])
                    cbq = cfg.CHUNKS[q]
                    if b % GBAT == 0:
                        t = b // GBAT
                        g0, g1 = int(coff[b]), int(coff[b + GBAT])
                        g_base = g0
                        nseg = g1 - g0
                        nidx = nseg * 128
                        msg = mp.tile([128, nseg * 512], fp8, tag="msg")
                        in_ap = xp_d[:] if layer == 1 else \
                            h1g[:].bitcast(fp8)
                        nc.gpsimd.dma_gather(
                            out_ap=msg[:].rearrange("p (g e) -> p g e",
                                                    e=512),
                            in_ap=in_ap,
                            idxs_ap=idxw_t[:, int(woff[t]):int(woff[t + 1])],
                            num_idxs=nidx,
                            num_idxs_reg=nidx,
                            elem_size=512,
                            single_packet=False)

                    cols = scol[b]
                    nbg = len(cols)
                    boff = int(coff[b]) - g_base

                    ind = ip.tile([128, nbg * 128], bf16, tag="ind")
                    for g in range(nbg):
                        c = int(coff[b]) + g
                        nc.vector.tensor_scalar(
                            ind[:, g * 128:(g + 1) * 128],
                            iota_t[:],
                            dstl_t[:, c:c + 1],
                            invc_t[:, c:c + 1],
                            AOT.is_equal, AOT.mult)

                    pa = pa_pool.tile([128, 128], f32, tag="agg")
                    for g in range(nbg):
                        mo = (boff + g) * 512 + cols[g] * 128
                        nc.tensor.matmul(
                            pa[:], msg[:, mo:mo + 128],
                            ind[:, g * 128:(g + 1) * 128],
                            start=(g == 0), stop=(g == nbg - 1))

                    aggsb = sp.tile([128, 128], bf16, tag="aggsb")
                    nc.scalar.activation(aggsb[:], pa[:], ACT_F.Copy,
                                         bias=0.0, scale=1.0)
                    if layer == 1:
                        # node-major h rows -> fp8 table for layer 2
                        ph = ph_pool.tile([128, 128], f32, tag="ph")
                        nc.tensor.matmul(ph[:], aggsb[:], wl_t[:],
                                         start=True, stop=False)
                        nc.tensor.matmul(
                            ph[:], xTs_t[:, b * 128:b * 128 + 128],
                            wr_t[:], start=False, stop=False)
                        nc.tensor.matmul(ph[:], ones_t[:], brow_t[:],
                                         start=False, stop=True)
                        hsb = sp.tile([128, 128], fp8, tag="hsb")
                        nc.scalar.activation(hsb[:], ph[:], ACT_F.Relu,
                                             bias=0.0, scale=1.0)
                        r0 = (b - b0) * 128
                        nc.sync.dma_start(
                            h1own[q][r0 // 4:(r0 + 128) // 4, :],
                            hsb[:].bitcast(f32))
                        # feat-major h -> exact bf16 self term for layer 2
                        ph2 = ph2_pool.tile([128, 128], f32, tag="ph2")
                        nc.tensor.matmul(ph2[:], wl_t[:], aggsb[:],
                                         start=True, stop=False)
                        nc.tensor.matmul(
                            ph2[:], wr_t[:],
                            xTs_t[:, b * 128:b * 128 + 128],
                            start=False, stop=False)
                        nc.tensor.matmul(ph2[:], brow_t[:], ones_t[:],
                                         start=False, stop=True)
                        nc.scalar.activation(
                            xT2_t[:, b * 128:b * 128 + 128], ph2[:],
                            ACT_F.Relu, bias=0.0, scale=1.0)
                        # chunk-q AllGather, emitted one gather-batch late
                        # so its sem wait (holding Pool SEQ) doesn't block
                        # the next chunk's gathers in the Pool queue.
                        def emit_coll(qq):
                            r0 = int(cfg.CUMR[qq]) * cfg.NC // 4
                            r1 = int(cfg.CUMR[qq + 1]) * cfg.NC // 4
                            nc.gpsimd.collective_compute(
                                "AllGather", AOT.bypass,
                                replica_groups=[list(range(cfg.NC))],
                                ins=[h1own[qq][:, :]],
                                outs=[h1g[r0:r1, :]])
                        if q > 0 and b - b0 == min(cbq - 1, 5):
                            emit_coll(q - 1)
                        if b == BLKS - 1:
                            emit_coll(cfg.NCOLL - 1)
                    else:
                        if b % 20 == 0:
                            lo_sb = dp.tile([1, 2560], f32, tag="lo")
                            hi_sb = dp.tile([1, 2560], f32, tag="hi")
                        off = (b % 20) * 128
                        # h2T = relu(wl.T@agg + wr.T@h1 + b) feat-major
                        ph2 = ph2_pool.tile([128, 128], f32, tag="ph2")
                        nc.tensor.matmul(ph2[:], wl_t[:], aggsb[:],
                                         start=True, stop=False)
                        nc.tensor.matmul(
                            ph2[:], wr_t[:],
                            xTs_t[:, b * 128:b * 128 + 128],
                            start=False, stop=False)
                        nc.tensor.matmul(ph2[:], brow_t[:], ones_t[:],
                                         start=False, stop=True)
                        h2T = sp.tile([128, 128], bf16, tag="h2T")
                        nc.scalar.activation(h2T[:], ph2[:], ACT_F.Relu,
                                             bias=0.0, scale=1.0)
                        pp = pd_pool.tile([1, 128], f32, tag="ppp")
                        nc.tensor.matmul(pp[:], wpd_t[:, 0:1], h2T[:],
                                         start=True, stop=False)
                        nc.tensor.matmul(pp[:], bpd_t[0:1, 0:1], ones_t[:],
                                         start=False, stop=True)
                        zz = pd_pool.tile([1, 128], f32, tag="zzp")
                        nc.tensor.matmul(zz[:], wpd_t[:, 1:2], h2T[:],
                                         start=True, stop=False)
                        nc.tensor.matmul(zz[:], bpd_t[0:1, 1:2], ones_t[:],
                                         start=False, stop=True)
                        drow = sp.tile([1, 128], f32, tag="drow")
                        nc.scalar.activation(drow[:], zz[:], ACT_F.Sigmoid,
                                             bias=0.0, scale=1.0)
                        nc.vector.tensor_sub(
                            lo_sb[0:1, off:off + 128], pp[:], drow[:])
                        nc.vector.tensor_add(
                            hi_sb[0:1, off:off + 128], pp[:], drow[:])
                        if b % 20 == 19:
                            c0 = (b - 19) * 128
                            nc.sync.dma_start(
                                pd_d[0:1, c0:c0 + 2560], lo_sb[:])
                            nc.sync.dma_start(
                                pd_d[1:2, c0:c0 + 2560], hi_sb[:])
    nc.compile()
    return nc


# ---------------- device path ----------------

_compiled = None       # (key, nc)


def _device_kernel(cfg, x, edge_index, Wl1, Wr1, b1, Wl2, Wr2, b2,
                   Wp, bp, Wd, bd):
    global _compiled, LAST_TRACE, LAST_LAUNCH_S
    import time as _time
    from concourse.bass_utils import run_bass_kernel_spmd

    prep = _prep(cfg, edge_index, x)
    key = prep["scol"]
    if _compiled is None or _compiled[0] != key:
        nc = _build(cfg, prep["scol"], prep["coff"], prep["woff"])
        _compiled = (key, nc)
    nc = _compiled[1]

    def bfw(a):
        return np.ascontiguousarray(np.asarray(a, np.float32).astype(BF16))

    bp_val = float(np.asarray(bp).reshape(-1)[0])
    bd_val = float(np.asarray(bd).reshape(-1)[0])
    wpd = np.stack([np.asarray(Wp, np.float32).reshape(-1),
                    np.asarray(Wd, np.float32).reshape(-1)], 1).astype(BF16)
    bpd = np.array([[bp_val, bd_val]], np.float32).astype(BF16)
    ones_r = np.ones((1, cfg.D), BF16)

    in_maps = []
    for k in range(cfg.NC):
        in_maps.append({
            "xpad": prep["xpad"],
            "xT1": prep["xT1"][k],
            "idxw": prep["idxw"][k],
            "dstl_all": prep["dstl_all"][k],
            "invc_all": prep["invc_all"][k],
            "iota128": prep["iota128"],
            "wl1": bfw(Wl1), "wr1": bfw(Wr1),
            "wl2": bfw(Wl2), "wr2": bfw(Wr2),
            "brow1": bfw(np.asarray(b1).reshape(1, cfg.D)),
            "brow2": bfw(np.asarray(b2).reshape(1, cfg.D)),
            "ones_r": ones_r,
            "wpd": wpd, "bpd": bpd,
        })

    trace = bool(os.environ.get("KERNEL_TRACE")) and _trace_available()
    _t0 = _time.time()
    res = run_bass_kernel_spmd(nc, in_maps, core_ids=list(range(cfg.NC)),
                               trace=trace)
    LAST_LAUNCH_S = _time.time() - _t0
    LAST_TRACE = res
    outs = res.results if hasattr(res, "results") else res
    pd = np.stack([np.asarray(o["pd"], np.float32) for o in outs])
    # unshard: node n lives at (core_of[n], li_of[n])
    core_of, li_of = prep["core_of"], prep["li_of"]
    lo = pd[core_of, 0, li_of].astype(np.float32).reshape(cfg.N, 1)
    hi = pd[core_of, 1, li_of].astype(np.float32).reshape(cfg.N, 1)
    return np.ascontiguousarray(lo), np.ascontiguousarray(hi)


# ---------------- host fallback ----------------


def _host_kernel(x, edge_index, Wl1, Wr1, b1, Wl2, Wr2, b2, Wp, bp, Wd, bd):
    N = CFG.N
    x = np.asarray(x, np.float32)
    src = np.asarray(edge_index[0], np.int64)
    dst = np.asarray(edge_index[1], np.int64)
    order = np.argsort(dst, kind="stable")
    src_s, dst_s = src[order], dst[order]
    counts = np.bincount(dst_s, minlength=N)
    starts = np.zeros(N, np.int64)
    starts[1:] = np.cumsum(counts)[:-1]
    nz = counts > 0
    inv = (1.0 / np.maximum(counts[nz], 1)).astype(np.float32)

    def mean_agg(f):
        sums = np.add.reduceat(f[src_s], starts[nz], axis=0)
        agg = np.zeros((N, f.shape[1]), np.float32)
        agg[nz] = sums * inv[:, None]
        return agg

    def layer(f, Wl, Wr, b):
        return np.maximum(mean_agg(f) @ Wl + f @ Wr + b, 0.0)

    h = layer(x, np.asarray(Wl1, np.float32), np.asarray(Wr1, np.float32),
              np.asarray(b1, np.float32))
    h = layer(h, np.asarray(Wl2, np.float32), np.asarray(Wr2, np.float32),
              np.asarray(b2, np.float32))
    preds = h @ np.asarray(Wp, np.float32) + np.asarray(bp, np.float32)
    z = h @ np.asarray(Wd, np.float32) + np.asarray(bd, np.float32)
    diffs = 1.0 / (1.0 + np.exp(-z))
    return ((preds - diffs).astype(np.float32),
            (preds + diffs).astype(np.float32))


# ---------------- entry ----------------


def kernel(x, edge_index, Wl1, Wr1, b1, Wl2, Wr2, b2, Wp, bp, Wd, bd):
    if not os.environ.get("KERNEL_HOST_ONLY"):
        try:
            return _device_kernel(CFG, x, edge_index, Wl1, Wr1, b1,
                                  Wl2, Wr2, b2, Wp, bp, Wd, bd)
        except Exception:
            import traceback
            traceback.print_exc()
    return _host_kernel(x, edge_index, Wl1, Wr1, b1, Wl2, Wr2, b2,
                        Wp, bp, Wd, bd)


# revision 24
# speedup vs baseline: 1.0497x; 1.0003x over previous
"""GraphSAGE 2-layer GNN, fully on-device on 8 Trainium2 NeuronCores.

Node-parallel sharding: dst nodes are greedily bin-packed into 800
blocks of 128 slots (100 blocks per core) so every block carries a
near-equal number of edges (~2000).  Messages are fetched with the
SWDGE `dma_gather` custom op (mlp gpsimd library): ONE instruction
per pair of blocks moves ~4200 rows (single_packet=False), instead of
one 128-row indirect DMA per 128 edges -- the per-instruction SWDGE
fixed cost dominated the old kernel.

dma_gather indices are int16, so the feature table is packed as QUADS:
[25600, 512] fp8(e3m4) rows holding 4 nodes each; an edge fetches its
src's quad (512B) and the matmul reads the right 128B quarter.  To
keep one matmul per 128-edge group, each block's edges are grouped by
their src's quad sub-slot ("color" = local-slot % 4), and a host-side
greedy assigns node colors to balance the per-(block, color) edge
counts; the per-block group->color pattern is shared by all 8 cores
(SPMD) and baked into the program.

Segment-mean aggregation on the tensor engine: per 128-edge group an
indicator ind[e, n] = (slot(dst[e]) == n) * (1/deg(dst[e])) is built
with ONE fused DVE tensor_scalar (in0 = stride-1 iota tile, scalar1 =
dst-slot column, scalar2 = 1/deg column, is_equal + mult -- 4x DVE
perf mode), then psum[feat, slot] += matmul(lhsT=msg quarter (fp8),
rhs=ind (bf16)).  PSUM drains and ReLUs run on the ACT engine.
Layer-1 h is produced in both orientations: node-major fp8 rows
written per block to HBM (the layer-2 message table, AllGather'd in 5
chunks; the tables are f32-typed because the runtime mishandles
1-byte collectives -- producers/consumers bitcast) and feat-major
bf16 in SBUF as the exact layer-2 self term.  Output heads are tiny
matmuls per block; lo/hi rows accumulate in SBUF, written back once
per 20-block chunk.

One Bacc program, compiled once, SPMD on cores 0-7; all
data-dependent quantities (gather rows, dst slots, 1/deg, per-block
color patterns) are inputs / compile keys.  A pure-numpy fallback is
kept in case the device path raises."""

import heapq
import os
import numpy as np
import ml_dtypes

BF16 = ml_dtypes.bfloat16
FP8 = ml_dtypes.float8_e3m4

# ---------------- configuration ----------------


class Cfg:
    def __init__(self, n_nodes, n_edges, n_cores, chunks, gbatch):
        self.N = n_nodes
        self.E = n_edges
        self.NC = n_cores
        self.D = 128
        self.OWNP = 12800              # node slots per core
        self.BLKS = self.OWNP // 128   # dst blocks per core
        self.NBINS = self.BLKS * n_cores
        self.PADN = self.OWNP * n_cores
        # collective chunk sizes in blocks: small first (starts the
        # serialized collective chain early) and small last (short tail)
        self.CHUNKS = list(chunks)
        assert sum(self.CHUNKS) == self.BLKS
        self.GBATCH = gbatch           # blocks per gather instruction
        assert all(c % gbatch == 0 for c in self.CHUNKS)
        self.NCOLL = len(self.CHUNKS)
        self.CUMB = np.cumsum([0] + self.CHUNKS)      # block boundaries
        self.CUMR = self.CUMB * 128                   # row boundaries


def _parse_chunks():
    s = os.environ.get("KERNEL_CHUNKS", "6,10,14,18,24,28")
    return [int(x) for x in s.split(",")]


CFG = Cfg(n_nodes=100000, n_edges=1600000, n_cores=8,
          chunks=_parse_chunks(),
          gbatch=int(os.environ.get("KERNEL_GBATCH", "2")))

LAST_TRACE = None     # BassKernelResults of the launch (test use)
LAST_LAUNCH_S = None  # wall seconds of the device launch (test use)


def _trace_available():
    try:
        from antenv.axon_hooks import get_axon_ntff_profile_hook  # noqa
        return True
    except Exception:
        return False


# ---------------- host-side edge prep ----------------


def _balance_bins(cfg, deg):
    """Greedy LPT: assign nodes to NBINS bins of <=128 nodes so per-bin
    edge (dst-degree) sums are near-equal.  Returns bin_of[node]."""
    order = np.argsort(-deg, kind="stable")
    bin_of = np.empty(cfg.N, np.int32)
    counts = np.zeros(cfg.NBINS, np.int32)
    heap = [(0, b) for b in range(cfg.NBINS)]
    heapq.heapify(heap)
    for n in order:
        while True:
            s, b = heapq.heappop(heap)
            if counts[b] < 128:
                break
        bin_of[n] = b
        counts[b] += 1
        if counts[b] < 128:
            heapq.heappush(heap, (s + int(deg[n]), b))
    return bin_of


def _color_assign(cfg, src, dst, bin_of):
    """Assign each node a color 0..3 (its quad sub-slot) balancing the
    per-(dst-bin, src-color) edge counts; <=32 nodes per (own-bin,
    color).  Returns c_of[node], slot_of[node] (slot%4 == color), L."""
    order = np.argsort(src, kind="stable")
    bins_o = bin_of[dst[order]]
    srcdeg = np.bincount(src, minlength=cfg.N)
    starts = np.zeros(cfg.N + 1, np.int64)
    starts[1:] = np.cumsum(srcdeg)

    L = np.zeros((cfg.NBINS, 4), np.int64)
    ccount = np.zeros((cfg.NBINS, 4), np.int32)
    c_of = np.full(cfg.N, -1, np.int8)
    slot_of = np.empty(cfg.N, np.int32)

    for n in np.argsort(-srcdeg, kind="stable"):
        own = bin_of[n]
        seg = bins_o[starts[n]:starts[n + 1]]
        if len(seg):
            bs, ks = np.unique(seg, return_counts=True)
            best_c, best_s = -1, None
            for c in range(4):
                if ccount[own, c] >= 32:
                    continue
                sc = np.sum((2 * L[bs, c] + ks) * ks)
                if best_s is None or sc < best_s:
                    best_c, best_s = c, sc
        else:
            best_c = int(np.argmin(
                np.where(ccount[own] < 32, ccount[own], 999)))
        c_of[n] = best_c
        slot_of[n] = 4 * ccount[own, best_c] + best_c
        ccount[own, best_c] += 1
        if len(seg):
            L[bs, best_c] += ks
    return c_of, slot_of, L


def _row_of(cfg, core, li):
    """HBM row of local slot li on core: chunk-major so each collective
    chunk's AllGather output is contiguous (variable chunk sizes)."""
    blk = li // 128
    q = np.searchsorted(cfg.CUMB, blk, side="right") - 1
    rows_q = (cfg.CUMB[q + 1] - cfg.CUMB[q]) * 128
    return cfg.CUMR[q] * cfg.NC + core * rows_q + (li - cfg.CUMR[q])


def _prep(cfg, edge_index, x):
    src = np.asarray(edge_index[0], np.int64)
    dst = np.asarray(edge_index[1], np.int64)

    deg = np.bincount(dst, minlength=cfg.N)
    invc_node = (1.0 / np.maximum(deg, 1)).astype(np.float32)

    bin_of = _balance_bins(cfg, deg)
    c_of, slot_of, L = _color_assign(cfg, src, dst, bin_of)
    core_of = bin_of // cfg.BLKS
    blk_of = bin_of % cfg.BLKS
    li_of = blk_of * 128 + slot_of              # local node slot per core
    row_of = _row_of(cfg, core_of, li_of).astype(np.int64)
    qrow_of = (row_of // 4).astype(np.int16)    # quad row (int16-safe)
    assert row_of.max() < cfg.PADN and (row_of // 4).max() < 32768

    # per-(block, color) group structure, shared across cores
    gcount = -(-L // 128)                       # [NBINS, 4] groups needed
    gshared = np.zeros((cfg.BLKS, 4), np.int64)
    for c in range(4):
        gshared[:, c] = gcount[:, c].reshape(cfg.NC, cfg.BLKS).max(0)
    gshared = np.maximum(gshared, 0)
    scol = []                                    # per blk: color per group
    coff = np.zeros(cfg.BLKS + 1, np.int64)
    cbase = np.zeros((cfg.BLKS, 4), np.int64)
    for blk in range(cfg.BLKS):
        cols = []
        off = coff[blk]
        for c in range(4):
            cbase[blk, c] = off
            cols += [c] * int(gshared[blk, c])
            off += gshared[blk, c]
        scol.append(tuple(cols))
        coff[blk + 1] = off
    totg = int(coff[-1])

    # place edges into (color-segmented) slots of their dst's bin
    be = bin_of[dst]
    ce = c_of[src].astype(np.int64)
    key = be * 4 + ce
    order = np.argsort(key, kind="stable")
    kcounts = np.bincount(key, minlength=cfg.NBINS * 4)
    kstarts = np.zeros(cfg.NBINS * 4, np.int64)
    kstarts[1:] = np.cumsum(kcounts)[:-1]
    rank = np.arange(cfg.E, dtype=np.int64) - kstarts[key[order]]
    so, do = src[order], dst[order]
    beo, ceo = be[order], ce[order]
    core_e = beo // cfg.BLKS
    col = cbase[beo % cfg.BLKS, ceo] + rank // 128
    p = rank % 128

    idx_lin = np.zeros((cfg.NC, totg, 128), np.int16)
    dstl_all = np.full((cfg.NC, 128, totg), -1.0, np.float32)
    invc_all = np.zeros((cfg.NC, 128, totg), np.float32)
    idx_lin[core_e, col, p] = qrow_of[so]
    dstl_all[core_e, p, col] = slot_of[do].astype(np.float32)
    invc_all[core_e, p, col] = invc_node[do]

    # wrap indices per gather instruction: blocks [t*GBATCH, (t+1)*GBATCH);
    # within an instruction, index r -> [r % 16 (replicated x8), r // 16]
    nins = cfg.BLKS // cfg.GBATCH
    woff = np.zeros(nins + 1, np.int64)
    for t in range(nins):
        nidx = int(coff[(t + 1) * cfg.GBATCH] - coff[t * cfg.GBATCH]) * 128
        woff[t + 1] = woff[t] + nidx // 16
    idxw = np.zeros((cfg.NC, 128, int(woff[-1])), np.int16)
    for t in range(nins):
        g0, g1 = int(coff[t * cfg.GBATCH]), int(coff[(t + 1) * cfg.GBATCH])
        seg = idx_lin[:, g0:g1, :].reshape(cfg.NC, -1)   # [NC, nidx]
        nidx = seg.shape[1]
        w = np.zeros((cfg.NC, 16, nidx // 16), np.int16)
        r = np.arange(nidx)
        w[:, r % 16, r // 16] = seg
        idxw[:, :, int(woff[t]):int(woff[t + 1])] = np.tile(w, (1, 8, 1))
    idxw = np.ascontiguousarray(idxw)

    x32 = np.asarray(x, np.float32)
    xpad = np.zeros((cfg.PADN, cfg.D), FP8)
    xpad[row_of] = x32.astype(FP8)
    xpad = np.ascontiguousarray(xpad.reshape(cfg.PADN // 4, cfg.D * 4))
    x_bf = np.ascontiguousarray(x32.astype(BF16))
    xT1 = np.zeros((cfg.NC, cfg.D, cfg.OWNP), BF16)
    for k in range(cfg.NC):
        sel = core_of == k
        xT1[k][:, li_of[sel]] = x_bf[sel].T

    iota128 = np.ascontiguousarray(
        np.broadcast_to(np.arange(128, dtype=np.float32).astype(BF16),
                        (128, 128)))

    return dict(scol=tuple(scol), totg=totg, coff=coff, woff=woff,
                idxw=idxw, dstl_all=dstl_all, invc_all=invc_all,
                xpad=xpad, xT1=xT1, iota128=iota128,
                core_of=core_of, li_of=li_of)


# ---------------- bass program ----------------


def _build(cfg, scol, coff, woff):
    import concourse.bass as bass  # noqa: F401
    import concourse.tile as tile
    import concourse.mybir as mybir
    from concourse import bacc
    from concourse.library_config import mlp

    f32 = mybir.dt.float32
    bf16 = mybir.dt.bfloat16
    fp8 = mybir.dt.float8e3
    i16 = mybir.dt.int16
    AOT = mybir.AluOpType
    ACT_F = mybir.ActivationFunctionType

    D, BLKS, GBAT = cfg.D, cfg.BLKS, cfg.GBATCH
    EMIT_OFF = int(os.environ.get("KERNEL_EMIT_OFF", "3"))
    totg = int(coff[-1])
    wtot = int(woff[-1])

    nc = bacc.Bacc("TRN2", target_bir_lowering=False, debug=False)
    xp_d = nc.dram_tensor("xpad", [cfg.PADN // 4, 4 * D], fp8,
                          kind="ExternalInput")
    xt1_d = nc.dram_tensor("xT1", [D, cfg.OWNP], bf16, kind="ExternalInput")
    idxw_d = nc.dram_tensor("idxw", [128, wtot], i16, kind="ExternalInput")
    dstl_d = nc.dram_tensor("dstl_all", [128, totg], f32,
                            kind="ExternalInput")
    invc_d = nc.dram_tensor("invc_all", [128, totg], f32,
                            kind="ExternalInput")
    iota_d = nc.dram_tensor("iota128", [128, 128], bf16, kind="ExternalInput")
    w_d = {}
    for w in ("wl1", "wr1", "wl2", "wr2"):
        w_d[w] = nc.dram_tensor(w, [D, D], bf16, kind="ExternalInput")
    brow1_d = nc.dram_tensor("brow1", [1, D], bf16, kind="ExternalInput")
    brow2_d = nc.dram_tensor("brow2", [1, D], bf16, kind="ExternalInput")
    ones_d = nc.dram_tensor("ones_r", [1, D], bf16, kind="ExternalInput")
    wpd_d = nc.dram_tensor("wpd", [D, 2], bf16, kind="ExternalInput")
    bpd_d = nc.dram_tensor("bpd", [1, 2], bf16, kind="ExternalInput")
    pd_d = nc.dram_tensor("pd", [2, cfg.OWNP], f32, kind="ExternalOutput")

    # h1 tables hold fp8 bytes but are typed f32: the runtime mishandles
    # 1-byte dtypes in collectives; producers/consumers bitcast.
    h1own = [nc.dram_tensor(f"h1own{q}", [cfg.CHUNKS[q] * 32, D], f32)
             for q in range(cfg.NCOLL)]
    h1g_shared = cfg.NC > 4 and not os.environ.get("KERNEL_H1G_LOCAL")
    h1g = nc.dram_tensor("h1gath", [cfg.PADN // 4, D], f32,
                         addr_space="Shared" if h1g_shared else "Local")

    with tile.TileContext(nc) as tc:
        with (
            tc.tile_pool(name="const", bufs=1) as cp,
            tc.tile_pool(name="msgp", bufs=3) as mp,
            tc.tile_pool(name="indp", bufs=3) as ip,
            tc.tile_pool(name="sbp", bufs=6) as sp,
            tc.tile_pool(name="pdp", bufs=2) as dp,
            tc.tile_pool(name="psa", bufs=2, space="PSUM") as pa_pool,
            tc.tile_pool(name="psh", bufs=2, space="PSUM") as ph_pool,
            tc.tile_pool(name="psh2", bufs=2, space="PSUM") as ph2_pool,
            tc.tile_pool(name="psd", bufs=1, space="PSUM") as pd_pool,
        ):
            nc.gpsimd.load_library(mlp)
            wt = {}
            for w in ("wl1", "wr1", "wl2", "wr2"):
                t = cp.tile([D, D], bf16, tag=w)
                nc.sync.dma_start(t[:], w_d[w][:])
                wt[w] = t
            brow1_t = cp.tile([1, D], bf16, tag="brow1")
            nc.sync.dma_start(brow1_t[:], brow1_d[:])
            brow2_t = cp.tile([1, D], bf16, tag="brow2")
            nc.sync.dma_start(brow2_t[:], brow2_d[:])
            ones_t = cp.tile([1, D], bf16, tag="ones")
            nc.sync.dma_start(ones_t[:], ones_d[:])
            wpd_t = cp.tile([D, 2], bf16, tag="wpd")
            nc.sync.dma_start(wpd_t[:], wpd_d[:])
            bpd_t = cp.tile([1, 2], bf16, tag="bpd")
            nc.sync.dma_start(bpd_t[:], bpd_d[:])
            iota_t = cp.tile([128, 128], bf16, tag="iota")
            nc.sync.dma_start(iota_t[:], iota_d[:])
            idxw_t = cp.tile([128, wtot], i16, tag="idxw")
            nc.sync.dma_start(idxw_t[:], idxw_d[:])
            dstl_t = cp.tile([128, totg], f32, tag="dstl")
            invc_t = cp.tile([128, totg], f32, tag="invc")
            xT1_t = cp.tile([D, cfg.OWNP], bf16, tag="xT1")
            xT2_t = cp.tile([D, cfg.OWNP], bf16, tag="xT2")
            # load only what the first PRE blocks need up front; the big
            # remainder loads are emitted after the first gather batch so
            # they queue behind it on the DMA engines
            PRE = 4 * GBAT
            pre_g = int(coff[PRE])
            nc.sync.dma_start(dstl_t[:, 0:pre_g], dstl_d[:, 0:pre_g])
            nc.sync.dma_start(invc_t[:, 0:pre_g], invc_d[:, 0:pre_g])
            nc.sync.dma_start(xT1_t[:, 0:PRE * 128], xt1_d[:, 0:PRE * 128])
            deferred = [(dstl_t, dstl_d, pre_g, totg),
                        (invc_t, invc_d, pre_g, totg),
                        (xT1_t, xt1_d, PRE * 128, cfg.OWNP)]

            for layer in (1, 2):
                wl_t = wt["wl1"] if layer == 1 else wt["wl2"]
                wr_t = wt["wr1"] if layer == 1 else wt["wr2"]
                brow_t = brow1_t if layer == 1 else brow2_t
                xTs_t = xT1_t if layer == 1 else xT2_t

                msg = None
                g_base = 0
                lo_sb = hi_sb = None
                for b in range(BLKS):
                    q = int(np.searchsorted(cfg.CUMB, b, side="right") - 1)
                    b0 = int(cfg.CUMB[q])
                    cbq = cfg.CHUNKS[q]
                    if layer == 1 and b == GBAT and deferred:
                        # big const loads deferred behind the first gather
                        for tt, dd, a, z in deferred:
                            nc.sync.dma_start(tt[:, a:z], dd[:, a:z])
                        deferred = []
                    if b % GBAT == 0:
                        t = b // GBAT
                        g0, g1 = int(coff[b]), int(coff[b + GBAT])
                        g_base = g0
                        nseg = g1 - g0
                        nidx = nseg * 128
                        msg = mp.tile([128, nseg * 512], fp8, tag="msg")
                        in_ap = xp_d[:] if layer == 1 else \
                            h1g[:].bitcast(fp8)
                        nc.gpsimd.dma_gather(
                            out_ap=msg[:].rearrange("p (g e) -> p g e",
                                                    e=512),
                            in_ap=in_ap,
                            idxs_ap=idxw_t[:, int(woff[t]):int(woff[t + 1])],
                            num_idxs=nidx,
                            num_idxs_reg=nidx,
                            elem_size=512,
                            single_packet=False)

                    cols = scol[b]
                    nbg = len(cols)
                    boff = int(coff[b]) - g_base

                    ind = ip.tile([128, nbg * 128], bf16, tag="ind")
                    for g in range(nbg):
                        c = int(coff[b]) + g
                        nc.vector.tensor_scalar(
                            ind[:, g * 128:(g + 1) * 128],
                            iota_t[:],
                            dstl_t[:, c:c + 1],
                            invc_t[:, c:c + 1],
                            AOT.is_equal, AOT.mult)

                    pa = pa_pool.tile([128, 128], f32, tag="agg")
                    for g in range(nbg):
                        mo = (boff + g) * 512 + cols[g] * 128
                        nc.tensor.matmul(
                            pa[:], msg[:, mo:mo + 128],
                            ind[:, g * 128:(g + 1) * 128],
                            start=(g == 0), stop=(g == nbg - 1))

                    aggsb = sp.tile([128, 128], bf16, tag="aggsb")
                    nc.scalar.activation(aggsb[:], pa[:], ACT_F.Copy,
                                         bias=0.0, scale=1.0)
                    if layer == 1:
                        # node-major h rows -> fp8 table for layer 2
                        ph = ph_pool.tile([128, 128], f32, tag="ph")
                        nc.tensor.matmul(ph[:], aggsb[:], wl_t[:],
                                         start=True, stop=False)
                        nc.tensor.matmul(
                            ph[:], xTs_t[:, b * 128:b * 128 + 128],
                            wr_t[:], start=False, stop=False)
                        nc.tensor.matmul(ph[:], ones_t[:], brow_t[:],
                                         start=False, stop=True)
                        hsb = sp.tile([128, 128], fp8, tag="hsb")
                        nc.scalar.activation(hsb[:], ph[:], ACT_F.Relu,
                                             bias=0.0, scale=1.0)
                        r0 = (b - b0) * 128
                        nc.sync.dma_start(
                            h1own[q][r0 // 4:(r0 + 128) // 4, :],
                            hsb[:].bitcast(f32))
                        # feat-major h -> exact bf16 self term for layer 2
                        ph2 = ph2_pool.tile([128, 128], f32, tag="ph2")
                        nc.tensor.matmul(ph2[:], wl_t[:], aggsb[:],
                                         start=True, stop=False)
                        nc.tensor.matmul(
                            ph2[:], wr_t[:],
                            xTs_t[:, b * 128:b * 128 + 128],
                            start=False, stop=False)
                        nc.tensor.matmul(ph2[:], brow_t[:], ones_t[:],
                                         start=False, stop=True)
                        nc.scalar.activation(
                            xT2_t[:, b * 128:b * 128 + 128], ph2[:],
                            ACT_F.Relu, bias=0.0, scale=1.0)
                        # chunk-q AllGather, emitted one gather-batch late
                        # so its sem wait (holding Pool SEQ) doesn't block
                        # the next chunk's gathers in the Pool queue.
                        def emit_coll(qq):
                            r0 = int(cfg.CUMR[qq]) * cfg.NC // 4
                            r1 = int(cfg.CUMR[qq + 1]) * cfg.NC // 4
                            nc.gpsimd.collective_compute(
                                "AllGather", AOT.bypass,
                                replica_groups=[list(range(cfg.NC))],
                                ins=[h1own[qq][:, :]],
                                outs=[h1g[r0:r1, :]])
                        if q > 0 and b - b0 == min(cbq - 1, EMIT_OFF):
                            emit_coll(q - 1)
                        if b == BLKS - 1:
                            emit_coll(cfg.NCOLL - 1)
                    else:
                        if b % 20 == 0:
                            lo_sb = dp.tile([1, 2560], f32, tag="lo")
                            hi_sb = dp.tile([1, 2560], f32, tag="hi")
                        off = (b % 20) * 128
                        # h2T = relu(wl.T@agg + wr.T@h1 + b) feat-major
                        ph2 = ph2_pool.tile([128, 128], f32, tag="ph2")
                        nc.tensor.matmul(ph2[:], wl_t[:], aggsb[:],
                                         start=True, stop=False)
                        nc.tensor.matmul(
                            ph2[:], wr_t[:],
                            xTs_t[:, b * 128:b * 128 + 128],
                            start=False, stop=False)
                        nc.tensor.matmul(ph2[:], brow_t[:], ones_t[:],
                                         start=False, stop=True)
                        h2T = sp.tile([128, 128], bf16, tag="h2T")
                        nc.scalar.activation(h2T[:], ph2[:], ACT_F.Relu,
                                             bias=0.0, scale=1.0)
                        pp = pd_pool.tile([1, 128], f32, tag="ppp")
                        nc.tensor.matmul(pp[:], wpd_t[:, 0:1], h2T[:],
                                         start=True, stop=False)
                        nc.tensor.matmul(pp[:], bpd_t[0:1, 0:1], ones_t[:],
                                         start=False, stop=True)
                        zz = pd_pool.tile([1, 128], f32, tag="zzp")
                        nc.tensor.matmul(zz[:], wpd_t[:, 1:2], h2T[:],
                                         start=True, stop=False)
                        nc.tensor.matmul(zz[:], bpd_t[0:1, 1:2], ones_t[:],
                                         start=False, stop=True)
                        drow = sp.tile([1, 128], f32, tag="drow")
                        nc.scalar.activation(drow[:], zz[:], ACT_F.Sigmoid,
                                             bias=0.0, scale=1.0)
                        nc.vector.tensor_sub(
                            lo_sb[0:1, off:off + 128], pp[:], drow[:])
                        nc.vector.tensor_add(
                            hi_sb[0:1, off:off + 128], pp[:], drow[:])
                        if b % 20 == 19:
                            c0 = (b - 19) * 128
                            nc.sync.dma_start(
                                pd_d[0:1, c0:c0 + 2560], lo_sb[:])
                            nc.sync.dma_start(
                                pd_d[1:2, c0:c0 + 2560], hi_sb[:])
    nc.compile()
    return nc


# ---------------- device path ----------------

_compiled = None       # (key, nc)


def _device_kernel(cfg, x, edge_index, Wl1, Wr1, b1, Wl2, Wr2, b2,
                   Wp, bp, Wd, bd):
    global _compiled, LAST_TRACE, LAST_LAUNCH_S
    import time as _time
    from concourse.bass_utils import run_bass_kernel_spmd

    prep = _prep(cfg, edge_index, x)
    key = prep["scol"]
    if _compiled is None or _compiled[0] != key:
        nc = _build(cfg, prep["scol"], prep["coff"], prep["woff"])
        _compiled = (key, nc)
    nc = _compiled[1]

    def bfw(a):
        return np.ascontiguousarray(np.asarray(a, np.float32).astype(BF16))

    bp_val = float(np.asarray(bp).reshape(-1)[0])
    bd_val = float(np.asarray(bd).reshape(-1)[0])
    wpd = np.stack([np.asarray(Wp, np.float32).reshape(-1),
                    np.asarray(Wd, np.float32).reshape(-1)], 1).astype(BF16)
    bpd = np.array([[bp_val, bd_val]], np.float32).astype(BF16)
    ones_r = np.ones((1, cfg.D), BF16)

    in_maps = []
    for k in range(cfg.NC):
        in_maps.append({
            "xpad": prep["xpad"],
            "xT1": prep["xT1"][k],
            "idxw": prep["idxw"][k],
            "dstl_all": prep["dstl_all"][k],
            "invc_all": prep["invc_all"][k],
            "iota128": prep["iota128"],
            "wl1": bfw(Wl1), "wr1": bfw(Wr1),
            "wl2": bfw(Wl2), "wr2": bfw(Wr2),
            "brow1": bfw(np.asarray(b1).reshape(1, cfg.D)),
            "brow2": bfw(np.asarray(b2).reshape(1, cfg.D)),
            "ones_r": ones_r,
            "wpd": wpd, "bpd": bpd,
        })

    trace = bool(os.environ.get("KERNEL_TRACE")) and _trace_available()
    _t0 = _time.time()
    res = run_bass_kernel_spmd(nc, in_maps, core_ids=list(range(cfg.NC)),
                               trace=trace)
    LAST_LAUNCH_S = _time.time() - _t0
    LAST_TRACE = res
    outs = res.results if hasattr(res, "results") else res
    pd = np.stack([np.asarray(o["pd"], np.float32) for o in outs])
    # unshard: node n lives at (core_of[n], li_of[n])
    core_of, li_of = prep["core_of"], prep["li_of"]
    lo = pd[core_of, 0, li_of].astype(np.float32).reshape(cfg.N, 1)
    hi = pd[core_of, 1, li_of].astype(np.float32).reshape(cfg.N, 1)
    return np.ascontiguousarray(lo), np.ascontiguousarray(hi)


# ---------------- host fallback ----------------


def _host_kernel(x, edge_index, Wl1, Wr1, b1, Wl2, Wr2, b2, Wp, bp, Wd, bd):
    N = CFG.N
    x = np.asarray(x, np.float32)
    src = np.asarray(edge_index[0], np.int64)
    dst = np.asarray(edge_index[1], np.int64)
    order = np.argsort(dst, kind="stable")
    src_s, dst_s = src[order], dst[order]
    counts = np.bincount(dst_s, minlength=N)
    starts = np.zeros(N, np.int64)
    starts[1:] = np.cumsum(counts)[:-1]
    nz = counts > 0
    inv = (1.0 / np.maximum(counts[nz], 1)).astype(np.float32)

    def mean_agg(f):
        sums = np.add.reduceat(f[src_s], starts[nz], axis=0)
        agg = np.zeros((N, f.shape[1]), np.float32)
        agg[nz] = sums * inv[:, None]
        return agg

    def layer(f, Wl, Wr, b):
        return np.maximum(mean_agg(f) @ Wl + f @ Wr + b, 0.0)

    h = layer(x, np.asarray(Wl1, np.float32), np.asarray(Wr1, np.float32),
              np.asarray(b1, np.float32))
    h = layer(h, np.asarray(Wl2, np.float32), np.asarray(Wr2, np.float32),
              np.asarray(b2, np.float32))
    preds = h @ np.asarray(Wp, np.float32) + np.asarray(bp, np.float32)
    z = h @ np.asarray(Wd, np.float32) + np.asarray(bd, np.float32)
    diffs = 1.0 / (1.0 + np.exp(-z))
    return ((preds - diffs).astype(np.float32),
            (preds + diffs).astype(np.float32))


# ---------------- entry ----------------


def kernel(x, edge_index, Wl1, Wr1, b1, Wl2, Wr2, b2, Wp, bp, Wd, bd):
    if not os.environ.get("KERNEL_HOST_ONLY"):
        try:
            return _device_kernel(CFG, x, edge_index, Wl1, Wr1, b1,
                                  Wl2, Wr2, b2, Wp, bp, Wd, bd)
        except Exception:
            import traceback
            traceback.print_exc()
    return _host_kernel(x, edge_index, Wl1, Wr1, b1, Wl2, Wr2, b2,
                        Wp, bp, Wd, bd)


# revision 27
# speedup vs baseline: 1.0528x; 1.0030x over previous
"""GraphSAGE 2-layer GNN, fully on-device on 8 Trainium2 NeuronCores.

Node-parallel sharding: dst nodes are greedily bin-packed into 800
blocks of 128 slots (100 blocks per core) so every block carries a
near-equal number of edges (~2000).  Messages are fetched with the
SWDGE `dma_gather` custom op (mlp gpsimd library): ONE instruction
per pair of blocks moves ~4200 rows (single_packet=False), instead of
one 128-row indirect DMA per 128 edges -- the per-instruction SWDGE
fixed cost dominated the old kernel.

dma_gather indices are int16, so the feature table is packed as QUADS:
[25600, 512] fp8(e3m4) rows holding 4 nodes each; an edge fetches its
src's quad (512B) and the matmul reads the right 128B quarter.  To
keep one matmul per 128-edge group, each block's edges are grouped by
their src's quad sub-slot ("color" = local-slot % 4), and a host-side
greedy assigns node colors to balance the per-(block, color) edge
counts; the per-block group->color pattern is shared by all 8 cores
(SPMD) and baked into the program.

Segment-mean aggregation on the tensor engine: per 128-edge group an
indicator ind[e, n] = (slot(dst[e]) == n) * (1/deg(dst[e])) is built
with ONE fused DVE tensor_scalar (in0 = stride-1 iota tile, scalar1 =
dst-slot column, scalar2 = 1/deg column, is_equal + mult -- 4x DVE
perf mode), then psum[feat, slot] += matmul(lhsT=msg quarter (fp8),
rhs=ind (bf16)).  PSUM drains and ReLUs run on the ACT engine.
Layer-1 h is produced in both orientations: node-major fp8 rows
written per block to HBM (the layer-2 message table, AllGather'd in 5
chunks; the tables are f32-typed because the runtime mishandles
1-byte collectives -- producers/consumers bitcast) and feat-major
bf16 in SBUF as the exact layer-2 self term.  Output heads are tiny
matmuls per block; lo/hi rows accumulate in SBUF, written back once
per 20-block chunk.

One Bacc program, compiled once, SPMD on cores 0-7; all
data-dependent quantities (gather rows, dst slots, 1/deg, per-block
color patterns) are inputs / compile keys.  A pure-numpy fallback is
kept in case the device path raises."""

import heapq
import os
import numpy as np
import ml_dtypes

BF16 = ml_dtypes.bfloat16
FP8 = ml_dtypes.float8_e3m4

# ---------------- configuration ----------------


class Cfg:
    def __init__(self, n_nodes, n_edges, n_cores, chunks, gbatch):
        self.N = n_nodes
        self.E = n_edges
        self.NC = n_cores
        self.D = 128
        self.OWNP = 12800              # node slots per core
        self.BLKS = self.OWNP // 128   # dst blocks per core
        self.NBINS = self.BLKS * n_cores
        self.PADN = self.OWNP * n_cores
        # collective chunk sizes in blocks: small first (starts the
        # serialized collective chain early) and small last (short tail)
        self.CHUNKS = list(chunks)
        assert sum(self.CHUNKS) == self.BLKS
        self.GBATCH = gbatch           # blocks per gather instruction
        assert all(c % gbatch == 0 for c in self.CHUNKS)
        self.NCOLL = len(self.CHUNKS)
        self.CUMB = np.cumsum([0] + self.CHUNKS)      # block boundaries
        self.CUMR = self.CUMB * 128                   # row boundaries


def _parse_chunks():
    s = os.environ.get("KERNEL_CHUNKS", "6,10,14,18,24,28")
    return [int(x) for x in s.split(",")]


CFG = Cfg(n_nodes=100000, n_edges=1600000, n_cores=8,
          chunks=_parse_chunks(),
          gbatch=int(os.environ.get("KERNEL_GBATCH", "2")))

LAST_TRACE = None     # BassKernelResults of the launch (test use)
LAST_LAUNCH_S = None  # wall seconds of the device launch (test use)


def _trace_available():
    try:
        from antenv.axon_hooks import get_axon_ntff_profile_hook  # noqa
        return True
    except Exception:
        return False


# ---------------- host-side edge prep ----------------


def _balance_bins(cfg, deg):
    """Greedy LPT: assign nodes to NBINS bins of <=128 nodes so per-bin
    edge (dst-degree) sums are near-equal.  Returns bin_of[node]."""
    order = np.argsort(-deg, kind="stable")
    bin_of = np.empty(cfg.N, np.int32)
    counts = np.zeros(cfg.NBINS, np.int32)
    heap = [(0, b) for b in range(cfg.NBINS)]
    heapq.heapify(heap)
    for n in order:
        while True:
            s, b = heapq.heappop(heap)
            if counts[b] < 128:
                break
        bin_of[n] = b
        counts[b] += 1
        if counts[b] < 128:
            heapq.heappush(heap, (s + int(deg[n]), b))
    return bin_of


def _color_assign(cfg, src, dst, bin_of):
    """Assign each node a color 0..3 (its quad sub-slot) balancing the
    per-(dst-bin, src-color) edge counts; <=32 nodes per (own-bin,
    color).  Returns c_of[node], slot_of[node] (slot%4 == color), L."""
    order = np.argsort(src, kind="stable")
    bins_o = bin_of[dst[order]]
    srcdeg = np.bincount(src, minlength=cfg.N)
    starts = np.zeros(cfg.N + 1, np.int64)
    starts[1:] = np.cumsum(srcdeg)

    L = np.zeros((cfg.NBINS, 4), np.int64)
    ccount = np.zeros((cfg.NBINS, 4), np.int32)
    c_of = np.full(cfg.N, -1, np.int8)
    slot_of = np.empty(cfg.N, np.int32)

    for n in np.argsort(-srcdeg, kind="stable"):
        own = bin_of[n]
        seg = bins_o[starts[n]:starts[n + 1]]
        if len(seg):
            bs, ks = np.unique(seg, return_counts=True)
            best_c, best_s = -1, None
            for c in range(4):
                if ccount[own, c] >= 32:
                    continue
                sc = np.sum((2 * L[bs, c] + ks) * ks)
                if best_s is None or sc < best_s:
                    best_c, best_s = c, sc
        else:
            best_c = int(np.argmin(
                np.where(ccount[own] < 32, ccount[own], 999)))
        c_of[n] = best_c
        slot_of[n] = 4 * ccount[own, best_c] + best_c
        ccount[own, best_c] += 1
        if len(seg):
            L[bs, best_c] += ks
    return c_of, slot_of, L


def _row_of(cfg, core, li):
    """HBM row of local slot li on core: chunk-major so each collective
    chunk's AllGather output is contiguous (variable chunk sizes)."""
    blk = li // 128
    q = np.searchsorted(cfg.CUMB, blk, side="right") - 1
    rows_q = (cfg.CUMB[q + 1] - cfg.CUMB[q]) * 128
    return cfg.CUMR[q] * cfg.NC + core * rows_q + (li - cfg.CUMR[q])


def _prep(cfg, edge_index, x):
    src = np.asarray(edge_index[0], np.int64)
    dst = np.asarray(edge_index[1], np.int64)

    deg = np.bincount(dst, minlength=cfg.N)
    invc_node = (1.0 / np.maximum(deg, 1)).astype(np.float32)

    bin_of = _balance_bins(cfg, deg)
    c_of, slot_of, L = _color_assign(cfg, src, dst, bin_of)
    core_of = bin_of // cfg.BLKS
    blk_of = bin_of % cfg.BLKS
    li_of = blk_of * 128 + slot_of              # local node slot per core
    row_of = _row_of(cfg, core_of, li_of).astype(np.int64)
    qrow_of = (row_of // 4).astype(np.int16)    # quad row (int16-safe)
    assert row_of.max() < cfg.PADN and (row_of // 4).max() < 32768

    # per-(block, color) group structure, shared across cores
    gcount = -(-L // 128)                       # [NBINS, 4] groups needed
    gshared = np.zeros((cfg.BLKS, 4), np.int64)
    for c in range(4):
        gshared[:, c] = gcount[:, c].reshape(cfg.NC, cfg.BLKS).max(0)
    gshared = np.maximum(gshared, 0)
    scol = []                                    # per blk: color per group
    coff = np.zeros(cfg.BLKS + 1, np.int64)
    cbase = np.zeros((cfg.BLKS, 4), np.int64)
    for blk in range(cfg.BLKS):
        cols = []
        off = coff[blk]
        for c in range(4):
            cbase[blk, c] = off
            cols += [c] * int(gshared[blk, c])
            off += gshared[blk, c]
        scol.append(tuple(cols))
        coff[blk + 1] = off
    totg = int(coff[-1])

    # place edges into (color-segmented) slots of their dst's bin
    be = bin_of[dst]
    ce = c_of[src].astype(np.int64)
    key = be * 4 + ce
    order = np.argsort(key, kind="stable")
    kcounts = np.bincount(key, minlength=cfg.NBINS * 4)
    kstarts = np.zeros(cfg.NBINS * 4, np.int64)
    kstarts[1:] = np.cumsum(kcounts)[:-1]
    rank = np.arange(cfg.E, dtype=np.int64) - kstarts[key[order]]
    so, do = src[order], dst[order]
    beo, ceo = be[order], ce[order]
    core_e = beo // cfg.BLKS
    col = cbase[beo % cfg.BLKS, ceo] + rank // 128
    p = rank % 128

    idx_lin = np.zeros((cfg.NC, totg, 128), np.int16)
    dstl_all = np.full((cfg.NC, 128, totg), -1.0, np.float32)
    invc_all = np.zeros((cfg.NC, 128, totg), np.float32)
    idx_lin[core_e, col, p] = qrow_of[so]
    dstl_all[core_e, p, col] = slot_of[do].astype(np.float32)
    invc_all[core_e, p, col] = invc_node[do]

    # wrap indices per gather instruction: blocks [t*GBATCH, (t+1)*GBATCH);
    # within an instruction, index r -> [r % 16 (replicated x8), r // 16]
    nins = cfg.BLKS // cfg.GBATCH
    woff = np.zeros(nins + 1, np.int64)
    for t in range(nins):
        nidx = int(coff[(t + 1) * cfg.GBATCH] - coff[t * cfg.GBATCH]) * 128
        woff[t + 1] = woff[t] + nidx // 16
    idxw = np.zeros((cfg.NC, 128, int(woff[-1])), np.int16)
    for t in range(nins):
        g0, g1 = int(coff[t * cfg.GBATCH]), int(coff[(t + 1) * cfg.GBATCH])
        seg = idx_lin[:, g0:g1, :].reshape(cfg.NC, -1)   # [NC, nidx]
        nidx = seg.shape[1]
        w = np.zeros((cfg.NC, 16, nidx // 16), np.int16)
        r = np.arange(nidx)
        w[:, r % 16, r // 16] = seg
        idxw[:, :, int(woff[t]):int(woff[t + 1])] = np.tile(w, (1, 8, 1))
    idxw = np.ascontiguousarray(idxw)

    x32 = np.asarray(x, np.float32)
    xpad = np.zeros((cfg.PADN, cfg.D), FP8)
    xpad[row_of] = x32.astype(FP8)
    xpad = np.ascontiguousarray(xpad.reshape(cfg.PADN // 4, cfg.D * 4))
    x_bf = np.ascontiguousarray(x32.astype(BF16))
    xT1 = np.zeros((cfg.NC, cfg.D, cfg.OWNP), BF16)
    for k in range(cfg.NC):
        sel = core_of == k
        xT1[k][:, li_of[sel]] = x_bf[sel].T

    iota128 = np.ascontiguousarray(
        np.broadcast_to(np.arange(128, dtype=np.float32).astype(BF16),
                        (128, 128)))

    return dict(scol=tuple(scol), totg=totg, coff=coff, woff=woff,
                idxw=idxw, dstl_all=dstl_all, invc_all=invc_all,
                xpad=xpad, xT1=xT1, iota128=iota128,
                core_of=core_of, li_of=li_of)


# ---------------- bass program ----------------


def _build(cfg, scol, coff, woff):
    import concourse.bass as bass  # noqa: F401
    import concourse.tile as tile
    import concourse.mybir as mybir
    from concourse import bacc
    from concourse.library_config import mlp

    f32 = mybir.dt.float32
    bf16 = mybir.dt.bfloat16
    fp8 = mybir.dt.float8e3
    i16 = mybir.dt.int16
    AOT = mybir.AluOpType
    ACT_F = mybir.ActivationFunctionType

    D, BLKS, GBAT = cfg.D, cfg.BLKS, cfg.GBATCH
    EMIT_OFF = int(os.environ.get("KERNEL_EMIT_OFF", "3"))
    totg = int(coff[-1])
    wtot = int(woff[-1])

    nc = bacc.Bacc("TRN2", target_bir_lowering=False, debug=False)
    xp_d = nc.dram_tensor("xpad", [cfg.PADN // 4, 4 * D], fp8,
                          kind="ExternalInput")
    xt1_d = nc.dram_tensor("xT1", [D, cfg.OWNP], bf16, kind="ExternalInput")
    idxw_d = nc.dram_tensor("idxw", [128, wtot], i16, kind="ExternalInput")
    dstl_d = nc.dram_tensor("dstl_all", [128, totg], f32,
                            kind="ExternalInput")
    invc_d = nc.dram_tensor("invc_all", [128, totg], f32,
                            kind="ExternalInput")
    iota_d = nc.dram_tensor("iota128", [128, 128], bf16, kind="ExternalInput")
    w_d = {}
    for w in ("wl1", "wr1", "wl2", "wr2"):
        w_d[w] = nc.dram_tensor(w, [D, D], bf16, kind="ExternalInput")
    brow1_d = nc.dram_tensor("brow1", [1, D], bf16, kind="ExternalInput")
    brow2_d = nc.dram_tensor("brow2", [1, D], bf16, kind="ExternalInput")
    ones_d = nc.dram_tensor("ones_r", [1, D], bf16, kind="ExternalInput")
    wpd_d = nc.dram_tensor("wpd", [D, 2], bf16, kind="ExternalInput")
    bpd_d = nc.dram_tensor("bpd", [1, 2], bf16, kind="ExternalInput")
    pd_d = nc.dram_tensor("pd", [2, cfg.OWNP], f32, kind="ExternalOutput")

    # h1 tables hold fp8 bytes but are typed f32: the runtime mishandles
    # 1-byte dtypes in collectives; producers/consumers bitcast.
    h1own = [nc.dram_tensor(f"h1own{q}", [cfg.CHUNKS[q] * 32, D], f32)
             for q in range(cfg.NCOLL)]
    h1g_shared = cfg.NC > 4 and not os.environ.get("KERNEL_H1G_LOCAL")
    h1g = nc.dram_tensor("h1gath", [cfg.PADN // 4, D], f32,
                         addr_space="Shared" if h1g_shared else "Local")

    with tile.TileContext(nc) as tc:
        with (
            tc.tile_pool(name="const", bufs=1) as cp,
            tc.tile_pool(name="msgp", bufs=3) as mp,
            tc.tile_pool(name="indp", bufs=3) as ip,
            tc.tile_pool(name="sbp", bufs=6) as sp,
            tc.tile_pool(name="pdp", bufs=2) as dp,
            tc.tile_pool(name="psa", bufs=2, space="PSUM") as pa_pool,
            tc.tile_pool(name="psh", bufs=2, space="PSUM") as ph_pool,
            tc.tile_pool(name="psh2", bufs=2, space="PSUM") as ph2_pool,
            tc.tile_pool(name="psd", bufs=1, space="PSUM") as pd_pool,
        ):
            nc.gpsimd.load_library(mlp)
            wt = {}
            for w in ("wl1", "wr1", "wl2", "wr2"):
                t = cp.tile([D, D], bf16, tag=w)
                wt[w] = t
            brow1_t = cp.tile([1, D], bf16, tag="brow1")
            brow2_t = cp.tile([1, D], bf16, tag="brow2")
            ones_t = cp.tile([1, D], bf16, tag="ones")
            wpd_t = cp.tile([D, 2], bf16, tag="wpd")
            bpd_t = cp.tile([1, 2], bf16, tag="bpd")
            iota_t = cp.tile([128, 128], bf16, tag="iota")
            idxw_t = cp.tile([128, wtot], i16, tag="idxw")
            dstl_t = cp.tile([128, totg], f32, tag="dstl")
            invc_t = cp.tile([128, totg], f32, tag="invc")
            xT1_t = cp.tile([D, cfg.OWNP], bf16, tag="xT1")
            xT2_t = cp.tile([D, cfg.OWNP], bf16, tag="xT2")
            # the first gather batch's index window plus the small consts
            # and short prefixes load up front; the three big remainders
            # are emitted after the first gather so they queue behind it
            # on the DMA engines
            w1 = int(woff[1])
            PRE = 4 * GBAT
            pre_g = int(coff[PRE])
            nc.sync.dma_start(idxw_t[:, 0:w1], idxw_d[:, 0:w1])
            for w in ("wl1", "wr1", "wl2", "wr2"):
                nc.sync.dma_start(wt[w][:], w_d[w][:])
            nc.sync.dma_start(brow1_t[:], brow1_d[:])
            nc.sync.dma_start(brow2_t[:], brow2_d[:])
            nc.sync.dma_start(ones_t[:], ones_d[:])
            nc.sync.dma_start(wpd_t[:], wpd_d[:])
            nc.sync.dma_start(bpd_t[:], bpd_d[:])
            nc.sync.dma_start(iota_t[:], iota_d[:])
            nc.sync.dma_start(dstl_t[:, 0:pre_g], dstl_d[:, 0:pre_g])
            nc.sync.dma_start(invc_t[:, 0:pre_g], invc_d[:, 0:pre_g])
            nc.sync.dma_start(xT1_t[:, 0:PRE * 128], xt1_d[:, 0:PRE * 128])

            def load_consts():
                nc.sync.dma_start(idxw_t[:, w1:], idxw_d[:, w1:])
                nc.sync.dma_start(dstl_t[:, pre_g:], dstl_d[:, pre_g:])
                nc.sync.dma_start(invc_t[:, pre_g:], invc_d[:, pre_g:])
                nc.sync.dma_start(xT1_t[:, PRE * 128:],
                                  xt1_d[:, PRE * 128:])
            deferred = True

            for layer in (1, 2):
                wl_t = wt["wl1"] if layer == 1 else wt["wl2"]
                wr_t = wt["wr1"] if layer == 1 else wt["wr2"]
                brow_t = brow1_t if layer == 1 else brow2_t
                xTs_t = xT1_t if layer == 1 else xT2_t

                msg = None
                g_base = 0
                lo_sb = hi_sb = None
                for b in range(BLKS):
                    q = int(np.searchsorted(cfg.CUMB, b, side="right") - 1)
                    b0 = int(cfg.CUMB[q])
                    cbq = cfg.CHUNKS[q]
                    if layer == 1 and b == GBAT and deferred:
                        load_consts()
                        deferred = False
                    if b % GBAT == 0:
                        t = b // GBAT
                        g0, g1 = int(coff[b]), int(coff[b + GBAT])
                        g_base = g0
                        nseg = g1 - g0
                        nidx = nseg * 128
                        msg = mp.tile([128, nseg * 512], fp8, tag="msg")
                        in_ap = xp_d[:] if layer == 1 else \
                            h1g[:].bitcast(fp8)
                        nc.gpsimd.dma_gather(
                            out_ap=msg[:].rearrange("p (g e) -> p g e",
                                                    e=512),
                            in_ap=in_ap,
                            idxs_ap=idxw_t[:, int(woff[t]):int(woff[t + 1])],
                            num_idxs=nidx,
                            num_idxs_reg=nidx,
                            elem_size=512,
                            single_packet=False)

                    cols = scol[b]
                    nbg = len(cols)
                    boff = int(coff[b]) - g_base

                    ind = ip.tile([128, nbg * 128], bf16, tag="ind")
                    for g in range(nbg):
                        c = int(coff[b]) + g
                        nc.vector.tensor_scalar(
                            ind[:, g * 128:(g + 1) * 128],
                            iota_t[:],
                            dstl_t[:, c:c + 1],
                            invc_t[:, c:c + 1],
                            AOT.is_equal, AOT.mult)

                    pa = pa_pool.tile([128, 128], f32, tag="agg")
                    for g in range(nbg):
                        mo = (boff + g) * 512 + cols[g] * 128
                        nc.tensor.matmul(
                            pa[:], msg[:, mo:mo + 128],
                            ind[:, g * 128:(g + 1) * 128],
                            start=(g == 0), stop=(g == nbg - 1))

                    aggsb = sp.tile([128, 128], bf16, tag="aggsb")
                    nc.scalar.activation(aggsb[:], pa[:], ACT_F.Copy,
                                         bias=0.0, scale=1.0)
                    if layer == 1:
                        # node-major h rows -> fp8 table for layer 2
                        ph = ph_pool.tile([128, 128], f32, tag="ph")
                        nc.tensor.matmul(ph[:], aggsb[:], wl_t[:],
                                         start=True, stop=False)
                        nc.tensor.matmul(
                            ph[:], xTs_t[:, b * 128:b * 128 + 128],
                            wr_t[:], start=False, stop=False)
                        nc.tensor.matmul(ph[:], ones_t[:], brow_t[:],
                                         start=False, stop=True)
                        hsb = sp.tile([128, 128], fp8, tag="hsb")
                        nc.scalar.activation(hsb[:], ph[:], ACT_F.Relu,
                                             bias=0.0, scale=1.0)
                        r0 = (b - b0) * 128
                        nc.sync.dma_start(
                            h1own[q][r0 // 4:(r0 + 128) // 4, :],
                            hsb[:].bitcast(f32))
                        # feat-major h -> exact bf16 self term for layer 2
                        ph2 = ph2_pool.tile([128, 128], f32, tag="ph2")
                        nc.tensor.matmul(ph2[:], wl_t[:], aggsb[:],
                                         start=True, stop=False)
                        nc.tensor.matmul(
                            ph2[:], wr_t[:],
                            xTs_t[:, b * 128:b * 128 + 128],
                            start=False, stop=False)
                        nc.tensor.matmul(ph2[:], brow_t[:], ones_t[:],
                                         start=False, stop=True)
                        nc.scalar.activation(
                            xT2_t[:, b * 128:b * 128 + 128], ph2[:],
                            ACT_F.Relu, bias=0.0, scale=1.0)
                        # chunk-q AllGather, emitted one gather-batch late
                        # so its sem wait (holding Pool SEQ) doesn't block
                        # the next chunk's gathers in the Pool queue.
                        def emit_coll(qq):
                            r0 = int(cfg.CUMR[qq]) * cfg.NC // 4
                            r1 = int(cfg.CUMR[qq + 1]) * cfg.NC // 4
                            nc.gpsimd.collective_compute(
                                "AllGather", AOT.bypass,
                                replica_groups=[list(range(cfg.NC))],
                                ins=[h1own[qq][:, :]],
                                outs=[h1g[r0:r1, :]])
                        if q > 0 and b - b0 == min(cbq - 1, EMIT_OFF):
                            emit_coll(q - 1)
                        if b == BLKS - 1:
                            emit_coll(cfg.NCOLL - 1)
                    else:
                        if b % 20 == 0:
                            lo_sb = dp.tile([1, 2560], f32, tag="lo")
                            hi_sb = dp.tile([1, 2560], f32, tag="hi")
                        off = (b % 20) * 128
                        # h2T = relu(wl.T@agg + wr.T@h1 + b) feat-major
                        ph2 = ph2_pool.tile([128, 128], f32, tag="ph2")
                        nc.tensor.matmul(ph2[:], wl_t[:], aggsb[:],
                                         start=True, stop=False)
                        nc.tensor.matmul(
                            ph2[:], wr_t[:],
                            xTs_t[:, b * 128:b * 128 + 128],
                            start=False, stop=False)
                        nc.tensor.matmul(ph2[:], brow_t[:], ones_t[:],
                                         start=False, stop=True)
                        h2T = sp.tile([128, 128], bf16, tag="h2T")
                        nc.scalar.activation(h2T[:], ph2[:], ACT_F.Relu,
                                             bias=0.0, scale=1.0)
                        pp = pd_pool.tile([1, 128], f32, tag="ppp")
                        nc.tensor.matmul(pp[:], wpd_t[:, 0:1], h2T[:],
                                         start=True, stop=False)
                        nc.tensor.matmul(pp[:], bpd_t[0:1, 0:1], ones_t[:],
                                         start=False, stop=True)
                        zz = pd_pool.tile([1, 128], f32, tag="zzp")
                        nc.tensor.matmul(zz[:], wpd_t[:, 1:2], h2T[:],
                                         start=True, stop=False)
                        nc.tensor.matmul(zz[:], bpd_t[0:1, 1:2], ones_t[:],
                                         start=False, stop=True)
                        drow = sp.tile([1, 128], f32, tag="drow")
                        nc.scalar.activation(drow[:], zz[:], ACT_F.Sigmoid,
                                             bias=0.0, scale=1.0)
                        nc.vector.tensor_sub(
                            lo_sb[0:1, off:off + 128], pp[:], drow[:])
                        nc.vector.tensor_add(
                            hi_sb[0:1, off:off + 128], pp[:], drow[:])
                        if b % 20 == 19:
                            c0 = (b - 19) * 128
                            nc.sync.dma_start(
                                pd_d[0:1, c0:c0 + 2560], lo_sb[:])
                            nc.sync.dma_start(
                                pd_d[1:2, c0:c0 + 2560], hi_sb[:])
    nc.compile()
    return nc


# ---------------- device path ----------------

_compiled = None       # (key, nc)


def _device_kernel(cfg, x, edge_index, Wl1, Wr1, b1, Wl2, Wr2, b2,
                   Wp, bp, Wd, bd):
    global _compiled, LAST_TRACE, LAST_LAUNCH_S
    import time as _time
    from concourse.bass_utils import run_bass_kernel_spmd

    prep = _prep(cfg, edge_index, x)
    key = prep["scol"]
    if _compiled is None or _compiled[0] != key:
        nc = _build(cfg, prep["scol"], prep["coff"], prep["woff"])
        _compiled = (key, nc)
    nc = _compiled[1]

    def bfw(a):
        return np.ascontiguousarray(np.asarray(a, np.float32).astype(BF16))

    bp_val = float(np.asarray(bp).reshape(-1)[0])
    bd_val = float(np.asarray(bd).reshape(-1)[0])
    wpd = np.stack([np.asarray(Wp, np.float32).reshape(-1),
                    np.asarray(Wd, np.float32).reshape(-1)], 1).astype(BF16)
    bpd = np.array([[bp_val, bd_val]], np.float32).astype(BF16)
    ones_r = np.ones((1, cfg.D), BF16)

    in_maps = []
    for k in range(cfg.NC):
        in_maps.append({
            "xpad": prep["xpad"],
            "xT1": prep["xT1"][k],
            "idxw": prep["idxw"][k],
            "dstl_all": prep["dstl_all"][k],
            "invc_all": prep["invc_all"][k],
            "iota128": prep["iota128"],
            "wl1": bfw(Wl1), "wr1": bfw(Wr1),
            "wl2": bfw(Wl2), "wr2": bfw(Wr2),
            "brow1": bfw(np.asarray(b1).reshape(1, cfg.D)),
            "brow2": bfw(np.asarray(b2).reshape(1, cfg.D)),
            "ones_r": ones_r,
            "wpd": wpd, "bpd": bpd,
        })

    trace = bool(os.environ.get("KERNEL_TRACE")) and _trace_available()
    _t0 = _time.time()
    res = run_bass_kernel_spmd(nc, in_maps, core_ids=list(range(cfg.NC)),
                               trace=trace)
    LAST_LAUNCH_S = _time.time() - _t0
    LAST_TRACE = res
    outs = res.results if hasattr(res, "results") else res
    pd = np.stack([np.asarray(o["pd"], np.float32) for o in outs])
    # unshard: node n lives at (core_of[n], li_of[n])
    core_of, li_of = prep["core_of"], prep["li_of"]
    lo = pd[core_of, 0, li_of].astype(np.float32).reshape(cfg.N, 1)
    hi = pd[core_of, 1, li_of].astype(np.float32).reshape(cfg.N, 1)
    return np.ascontiguousarray(lo), np.ascontiguousarray(hi)


# ---------------- host fallback ----------------


def _host_kernel(x, edge_index, Wl1, Wr1, b1, Wl2, Wr2, b2, Wp, bp, Wd, bd):
    N = CFG.N
    x = np.asarray(x, np.float32)
    src = np.asarray(edge_index[0], np.int64)
    dst = np.asarray(edge_index[1], np.int64)
    order = np.argsort(dst, kind="stable")
    src_s, dst_s = src[order], dst[order]
    counts = np.bincount(dst_s, minlength=N)
    starts = np.zeros(N, np.int64)
    starts[1:] = np.cumsum(counts)[:-1]
    nz = counts > 0
    inv = (1.0 / np.maximum(counts[nz], 1)).astype(np.float32)

    def mean_agg(f):
        sums = np.add.reduceat(f[src_s], starts[nz], axis=0)
        agg = np.zeros((N, f.shape[1]), np.float32)
        agg[nz] = sums * inv[:, None]
        return agg

    def layer(f, Wl, Wr, b):
        return np.maximum(mean_agg(f) @ Wl + f @ Wr + b, 0.0)

    h = layer(x, np.asarray(Wl1, np.float32), np.asarray(Wr1, np.float32),
              np.asarray(b1, np.float32))
    h = layer(h, np.asarray(Wl2, np.float32), np.asarray(Wr2, np.float32),
              np.asarray(b2, np.float32))
    preds = h @ np.asarray(Wp, np.float32) + np.asarray(bp, np.float32)
    z = h @ np.asarray(Wd, np.float32) + np.asarray(bd, np.float32)
    diffs = 1.0 / (1.0 + np.exp(-z))
    return ((preds - diffs).astype(np.float32),
            (preds + diffs).astype(np.float32))


# ---------------- entry ----------------


def kernel(x, edge_index, Wl1, Wr1, b1, Wl2, Wr2, b2, Wp, bp, Wd, bd):
    if not os.environ.get("KERNEL_HOST_ONLY"):
        try:
            return _device_kernel(CFG, x, edge_index, Wl1, Wr1, b1,
                                  Wl2, Wr2, b2, Wp, bp, Wd, bd)
        except Exception:
            import traceback
            traceback.print_exc()
    return _host_kernel(x, edge_index, Wl1, Wr1, b1, Wl2, Wr2, b2,
                        Wp, bp, Wd, bd)
